# revision 1
# baseline (speedup 1.0000x reference)
"""CRF marginal kernel for Trainium2 (8 NeuronCores, SPMD data-parallel over batch).

Reference math (keras_contrib CRF get_marginal_prob):
  e = X @ W + bias  (+ left/right boundary at t=0 / t=T-1)
  alpha/beta: logsumexp scans over T with transition chain[i,j]
  out = softmax_j(-(alpha_sr + e + beta_sl))

Kernel algorithm (per core, B_local=8):
  Linear-domain recurrence with constant per-step rescale c folded into the
  transition weights E'[i,j] = exp(-chain[i,j] - c):
      v_{t+1} = E'^T (v_t * Q_t),   Q_t = exp(-e_t),  v_0 = 1   (fwd)
  and the mirrored bwd scan. Per-(b,t) scale factors cancel in the final
  softmax, so each scan is split into H=8 segments run CONCURRENTLY, each
  burned in BURN=32 steps from an arbitrary init (the transition matrices are
  strongly mixing, so segments converge to the true state direction well
  within the burn-in; per-segment scale again cancels). Serial chain length
  drops 512 -> 96 steps. Each step is one [128,128] tile: 2 dirs x 8 segs x
  8 batch; 2 ops on the critical path (DVE multiply + PE matmul).

  Final combine, entirely from stored per-step q = state*Q:
      u[j,t,b] = Q_t^3 / (qf_t * qb_t)   (= exp(-(alpha_sr+e+beta_sl)) up to
  per-(b,t) scale), out = u / sum_j u via PE transpose + row softmax.

  Energy matmul: X loaded in time-stripe order matching recurrence
  consumption, X^T on-chip via PE transposes, fp16 matmuls (N=256), exp
  fused into the PSUM->SBUF eviction on ACT with bias/boundary folded in.
"""

import numpy as np

B, T, D, F = 64, 512, 2048, 128
NCORES = 8
BL = B // NCORES  # 8 batch per core
H = 8  # segments per scan direction
SEG = T // H  # 64
BURN = 16  # burn-in steps per segment (converges to fp32 floor; see burnin_check)
NSTEP = SEG + BURN  # 96 tile-steps; muls k=0..95, matmuls k=0..94
NSC = 16  # phase-1 super-chunks (4 time-stripes each)
PAD = BURN * BL  # 256 pad cols each side of QBUF
CSCALE = 5.3513  # mean per-step log-drift (concentration-stable statistic)


def build_nc():
    import concourse.bass as bass
    import concourse.mybir as mybir
    from concourse.tile import TileContext
    from concourse.ap import AP

    fp32 = mybir.dt.float32
    fp16 = mybir.dt.float16
    Act = mybir.ActivationFunctionType
    Alu = mybir.AluOpType

    nc = bass.Bass()
    Xd = nc.declare_dram_parameter("x", [BL, T, D], fp32, isOutput=False)
    Wd = nc.declare_dram_parameter("w", [D, F], fp32, isOutput=False)
    EWd = nc.declare_dram_parameter("ew", [F, F], fp32, isOutput=False)
    NBd = nc.declare_dram_parameter("nb", [F, 4], fp32, isOutput=False)
    IDd = nc.declare_dram_parameter("idn", [F, F], fp32, isOutput=False)
    OUTd = nc.declare_dram_parameter("out", [BL, T, F], fp32, isOutput=True)

    def sub(base, col_off, dims):
        """Custom free-dim AP into a [128, N] SBUF/PSUM tile view."""
        return AP(
            tensor=base.tensor,
            offset=base.offset + col_off,
            ap=[list(base.ap[0])] + [list(d) for d in dims],
        )

    def qcol(t):  # QBUF column of (t, b=0)
        return PAD + t * BL

    def pump(ap):
        """PE observation pump: a 1-col ldweights with a genuine cross-
        engine data dep. PE matmuls have a single sync-wait slot in
        walrus codegen; this absorbs one producer's wait so the real
        matmul that follows carries at most one."""
        if ap.dtype != fp16:
            ap = ap.bitcast(fp16)
        nc.tensor.ldweights(ap)

    with TileContext(nc) as tc:
        with (
            tc.tile_pool(name="const", bufs=1) as constp,
            tc.tile_pool(name="big", bufs=1) as bigp,
            tc.tile_pool(name="state", bufs=4, space="PSUM") as statep,
        ):
            # ---- constants ----
            w_sb = constp.tile([128, 16 * 128], fp32, name="w_sb")
            nc.sync.dma_start(
                out=w_sb[:].rearrange("p (c j) -> p c j", c=16),
                in_=Wd[:].rearrange("(c p) j -> p c j", p=128),
            )
            # ew/id go through DVE copies so PE consumers coalesce their
            # wait with other DVE deps (PE matmuls have ONE sync-wait slot).
            ew_ld = constp.tile([128, 128], fp32, name="ew_ld")
            nc.sync.dma_start(out=ew_ld[:], in_=EWd[:])
            ew_sb = constp.tile([128, 128], fp32, name="ew_sb")
            nc.vector.tensor_copy(ew_sb[:], ew_ld[:])
            id_ld = constp.tile([128, 128], fp32, name="id_ld")
            nc.sync.dma_start(out=id_ld[:], in_=IDd[:])
            id_sb = constp.tile([128, 128], fp32, name="id_sb")
            nc.vector.tensor_copy(id_sb[:], id_ld[:])
            # fp16 copy of W for the full-rate energy matmul
            w16 = constp.tile([128, 16 * 128], fp16, name="w16")
            nc.scalar.copy(w16[:], w_sb[:])
            nb_sb = constp.tile([128, 4], fp32, name="nb_sb")
            nc.sync.dma_start(out=nb_sb[:], in_=NBd[:])

            # ---- persistent big buffers ----
            # QBUF[:, PAD + t*8 + b] = exp(-e[b,t,:]); PAD cols of 1.0 each side
            qbuf = bigp.tile([128, 2 * PAD + T * BL], fp32, name="qbuf")
            nc.vector.memset(qbuf[:, :PAD], 1.0)
            nc.vector.memset(qbuf[:, PAD + T * BL :], 1.0)
            # QSTORE step-k tile at cols [k*128, (k+1)*128):
            #   col k*128 + g*8 + b         = fwd seg g
            #   col k*128 + 64 + i*8 + b    = bwd seg j=7-i
            qstore = bigp.tile([128, NSTEP * 128], fp32, name="qstore")
            # combine output staging: block t0 at cols (t0//16)*128, part j? no:
            # partitions = (b*16+dt) rows, free = j per block
            obuf = bigp.tile([128, (T // 16) * 128], fp32, name="obuf")
            scr = bigp.tile([128, 2], fp32, name="scr")
            scrp = bigp.tile([128, 64], fp32, name="scrp")  # 2 cols/block
            scrq = bigp.tile([128, NSTEP * 16], fp32, name="scrq")
            scrs = bigp.tile([128, NSTEP], fp32, name="scrs")
            scrc = bigp.tile([128, 2 * (T // 16)], fp32, name="scrc")

            prev_ps = None

            def emit_step(k):
                nonlocal prev_ps
                # fwd seg g at t = g*64 - 32 + k -> col qcol(k-32) + g*512
                # bwd block i (seg j=7-i) at t = 95 + 64*i - k
                offF = qcol(k - BURN)
                offB = qcol(SEG + BURN - 1 - k)
                qin = sub(
                    qbuf, offF, [[offB - offF, 2], [SEG * BL, H], [1, BL]]
                )
                qout = sub(qstore, k * 128, [[64, 2], [8, H], [1, BL]])
                # DVE pump: sample one col of every Q block the mul reads so
                # the single coalesced ACT wait lands here, not on the mul
                qsamp = sub(qbuf, offF, [[offB - offF, 2], [SEG * BL, H], [1, 1]])
                nc.vector.tensor_copy(
                    sub(scrq, k * 16, [[8, 2], [1, H], [1, 1]]), qsamp
                )
                if k == 0:
                    nc.vector.tensor_copy(qout, qin)
                else:
                    # DVE pump: absorb the PSUM-state (PE) wait so the mul
                    # carries only the single coalesced ACT wait for QBUF
                    nc.vector.tensor_copy(scrs[:, k : k + 1], sub(prev_ps, 0, [[1, 1]]))
                    pin = sub(prev_ps, 0, [[64, 2], [8, H], [1, BL]])
                    nc.vector.tensor_tensor(qout, pin, qin, op=Alu.mult)
                if k == BURN:
                    # exact init: fwd seg0 q = Q_{t=0}, bwd seg0 (block 7) = Q_{T-1}
                    ow_out = sub(qstore, k * 128, [[120, 2], [1, BL]])
                    ow_in = sub(qbuf, qcol(0), [[qcol(T - 1) - qcol(0), 2], [1, BL]])
                    nc.vector.tensor_copy(ow_out, ow_in)
                if k < NSTEP - 1:
                    ps = statep.tile([128, 128], fp32, name="st")
                    pump(sub(qstore, k * 128, [[1, 2]]))
                    nc.tensor.matmul(
                        ps[:],
                        ew_sb[:],
                        qstore[:, k * 128 : (k + 1) * 128],
                        start=True,
                        stop=True,
                    )
                    prev_ps = ps

            # ---------------- phase 1 (+ steps it unblocks) ----------------
            with (
                tc.tile_pool(name="xrow", bufs=4) as xrowp,
                tc.tile_pool(name="xtp", bufs=3) as xtp,
                tc.tile_pool(name="ptp", bufs=2, space="PSUM") as ptp,
                tc.tile_pool(name="pep", bufs=2, space="PSUM") as pep,
            ):
                # PE warmup: absorb the id_sb DVE-copy dependency into one
                # throwaway transpose so real transposes only wait the X DMA.
                warm = ptp.tile([128, 512], fp32, name="pt")
                nc.tensor.transpose(warm[:, 0:128], id_sb[:], id_sb[:])
                last_copy_dst = None
                for s in range(NSC):
                    # stripe production order matched to step consumption:
                    # chunks 0-7 make stripes {48..63, 0..15} (steps 0..15),
                    # chunks 8-15 make stripes {16..47} (steps 32..47 resume
                    # progressively; steps 16..31 reuse chunks <= 7).
                    if s < 8:
                        rpairs = ((48 + 2 * s, 49 + 2 * s), (14 - 2 * s, 15 - 2 * s))
                    else:
                        m = s - 8
                        rpairs = ((16 + 2 * m, 17 + 2 * m), (46 - 2 * m, 47 - 2 * m))
                    xt = xtp.tile([128, 16 * 256], fp16, name="xt")
                    xrows = []
                    for wi, (r0, _r1) in enumerate(rpairs):
                        xrow = xrowp.tile([128, D], fp32, name="xrow")
                        # rows (b, m, t2): t = r0 + t2 + 64*m
                        xin = AP(
                            tensor=Xd,
                            offset=r0 * D,
                            ap=[[T * D, BL], [SEG * D, 8], [D, 2], [1, D]],
                        )
                        # SWDGE (gpsimd) keeps the whole load on ONE sem so
                        # the consuming PE transposes carry a single wait
                        nc.gpsimd.dma_start(out=xrow[:], in_=xin)
                        xrows.append(xrow)
                    pe = pep.tile([128, 256], fp32, name="pe")

                    def tgroup(wi, dq):
                        # 4 transposes into one PSUM bank + one wide ACT copy
                        pump(xrows[wi][:, dq * 512 : dq * 512 + 2])
                        pt = ptp.tile([128, 512], fp32, name="pt")
                        for q in range(4):
                            d = dq * 4 + q
                            nc.tensor.transpose(
                                pt[:, q * 128 : (q + 1) * 128],
                                xrows[wi][:, d * 128 : (d + 1) * 128],
                                id_sb[:],
                            )
                        dst = sub(
                            xt, (dq * 4) * 256 + wi * 128, [[256, 4], [1, 128]]
                        )
                        nc.scalar.copy(dst, pt[:].rearrange("p (a b) -> p a b", a=4))

                    def mmq(dq):
                        # energy matmuls for d in [4dq, 4dq+4); their ACT waits
                        # advance PE's observed ACT tick so later transpose
                        # groups' WAR deps on older copies are covered
                        pump(xt[:, dq * 4 * 256 : dq * 4 * 256 + 2])
                        for d in range(4 * dq, 4 * dq + 4):
                            nc.tensor.matmul(
                                pe[:],
                                w16[:, d * 128 : (d + 1) * 128],
                                xt[:, d * 256 : (d + 1) * 256],
                                start=(d == 0),
                                stop=(d == 15),
                            )

                    tgroup(0, 0)
                    tgroup(0, 1)
                    tgroup(1, 0)
                    mmq(0)
                    tgroup(0, 2)
                    tgroup(1, 1)
                    mmq(1)
                    tgroup(0, 3)
                    tgroup(1, 2)
                    mmq(2)
                    tgroup(1, 3)
                    mmq(3)
                    # fused exp: PSUM -> QBUF. psum col = wi*128 + b*16 + m*2 + t2
                    # Q col = qcol(r0 + t2 + 64m) + b
                    for wi, (r0, r1) in enumerate(rpairs):
                        # (bias_col, m0, nm, t2_0, nt2)
                        segs = [(1, 0, 8, 0, 2)]
                        if s == 7 and wi == 1:  # pair (0,1): t=0 at (m=0,t2=0)
                            segs = [(0, 0, 1, 0, 1), (1, 0, 1, 1, 1), (1, 1, 7, 0, 2)]
                        if s == 7 and wi == 0:  # pair (62,63): t=511 at (m=7,t2=1)
                            segs = [(1, 0, 7, 0, 2), (1, 7, 1, 0, 1), (2, 7, 1, 1, 1)]
                        for bcol, m0, nm, t20, nt2 in segs:
                            pin = sub(
                                pe,
                                wi * 128 + m0 * 2 + t20,
                                [[16, BL], [2, nm], [1, nt2]],
                            )
                            qo = sub(
                                qbuf,
                                qcol(r0 + t20 + SEG * m0),
                                [[1, BL], [SEG * BL, nm], [BL, nt2]],
                            )
                            nc.scalar.activation(
                                qo,
                                pin,
                                Act.Exp,
                                bias=nb_sb[:, bcol : bcol + 1],
                                scale=-1.0,
                            )
                    if s < 8:
                        emit_step(2 * s)
                        emit_step(2 * s + 1)
                    else:
                        for k in range(16 + 4 * (s - 8), 20 + 4 * (s - 8)):
                            emit_step(k)

            # ---------------- rest of recurrence + combine ----------------
            with (
                tc.tile_pool(name="comb", bufs=3) as combp,
                tc.tile_pool(name="pup", bufs=2, space="PSUM") as pup,
            ):

                def emit_combine(t0):
                    # block covers t in [t0, t0+16): 128 cols ordered (b, dt)
                    # so the transposed output rows give 8KB-contiguous
                    # per-batch runs for the out-DMA.
                    g = t0 // SEG
                    j = (T - 16 - t0) // SEG  # bwd seg owning these t
                    qf = sub(
                        qstore,
                        (t0 - SEG * g + BURN) * 128 + g * 8,
                        [[1, BL], [128, 16]],
                    )
                    qb = sub(
                        qstore,
                        (T - 1 - t0 - SEG * j + BURN) * 128 + 64 + (H - 1 - j) * 8,
                        [[1, BL], [-128, 16]],
                    )
                    # SBUF-only elementwise work goes to GpSimd (idle engine)
                    # to keep DVE free for the recurrence chain.
                    mb = combp.tile([128, 128], fp32, name="mb")
                    bi = t0 // 16
                    qf_last = (t0 + 15 - SEG * g + BURN) * 128 + g * 8
                    qb_last = (T - 1 - t0 - SEG * j + BURN) * 128 + 64 + (H - 1 - j) * 8
                    qfs = sub(qstore, qf_last, [[qb_last - qf_last, 2], [1, 1]])
                    nc.gpsimd.tensor_copy(
                        sub(scrp, 2 * bi, [[1, 2], [1, 1]]), qfs
                    )
                    nc.gpsimd.tensor_tensor(
                        mb[:].rearrange("p (b a) -> p b a", b=BL), qf, qb, op=Alu.mult
                    )
                    rb = combp.tile([128, 128], fp32, name="rb")
                    nc.vector.tensor_copy(scrc[:, 2 * bi : 2 * bi + 1], mb[:, 0:1])
                    nc.vector.reciprocal(rb[:], mb[:])
                    qs = sub(qbuf, qcol(t0), [[1, BL], [BL, 16]])
                    q2 = combp.tile([128, 128], fp32, name="q2")
                    nc.gpsimd.tensor_tensor(
                        q2[:].rearrange("p (b a) -> p b a", b=BL), qs, qs, op=Alu.mult
                    )
                    q3 = combp.tile([128, 128], fp32, name="q3")
                    nc.gpsimd.tensor_tensor(
                        q3[:].rearrange("p (b a) -> p b a", b=BL),
                        q2[:].rearrange("p (b a) -> p b a", b=BL),
                        qs,
                        op=Alu.mult,
                    )
                    # ub on DVE and ut copy on DVE: the PE transpose then sees
                    # a single (coalesced) DVE wait for both input and WAR.
                    ub = combp.tile([128, 128], fp32, name="ub")
                    nc.vector.tensor_copy(scrc[:, 2 * bi + 1 : 2 * bi + 2], q3[:, 0:1])
                    nc.vector.tensor_tensor(ub[:], q3[:], rb[:], op=Alu.mult)
                    pu = pup.tile([128, 128], fp32, name="pu")
                    pump(ub[:, 0:2])
                    nc.tensor.transpose(pu[:], ub[:], id_sb[:])
                    ut = combp.tile([128, 128], fp32, name="ut")
                    nc.vector.tensor_copy(ut[:], pu[:])
                    sm = combp.tile([128, 1], fp32, name="sm")
                    nc.vector.reduce_sum(sm[:], ut[:], axis=mybir.AxisListType.X)
                    rs = combp.tile([128, 1], fp32, name="rs")
                    nc.vector.reciprocal(rs[:], sm[:])
                    ob = obuf[:, (t0 // 16) * 128 : (t0 // 16) * 128 + 128]
                    nc.vector.tensor_scalar_mul(ob, ut[:], rs[:])
                    # rows b*16+dt -> OUT[b, t0+dt, :]
                    oap = AP(
                        tensor=OUTd,
                        offset=t0 * F,
                        ap=[[T * F, BL], [F, 16], [1, F]],
                    )
                    emit_combine.n += 1
                    eng = (nc.sync, nc.gpsimd)[emit_combine.n % 2]
                    eng.dma_start(out=oap, in_=ob)

                # block t0 ready after step max((t0%64)+47, ((T-1-t0)%64)+32);
                # emit at most 2 per step so combine work doesn't head-of-line
                # block the DVE recurrence chain.
                ready = {}
                for blk in range(T // 16):
                    t0 = blk * 16
                    kf = (t0 % SEG) + BURN + 15
                    kb = ((T - 1 - t0) % SEG) + BURN
                    ready.setdefault(max(kf, kb), []).append(t0)

                emit_combine.n = 0
                pending = []
                for k in range(3 * NSC, NSTEP):
                    emit_step(k)
                    pending.extend(ready.get(k, []))
                    for _ in range(min(2, len(pending))):
                        emit_combine(pending.pop(0))
                for t0 in pending:
                    emit_combine(t0)

    _strip_redundant_waits(nc)
    return nc


def _strip_redundant_waits(nc):
    """Drop sync waits that hardware ordering already guarantees, to fit
    walrus's one-sync-wait-per-instruction limit on PE/DMA instructions:
    - PE->PE PSUM WAW waits: PE completions are pc-monotone (documented:
      a single then_inc on the last of concurrent MMs is sound), so an
      earlier PE write always lands before a later one.
    - SWDGE->SWDGE DMA WAW waits: mainline gpsimd DMAs share one physical
      FIFO queue (qPoolDynamic), so they complete in issue order.
    """
    import concourse.mybir as mybir

    for f in nc.m.functions:
        for bb in f.blocks:
            for inst in bb.instructions:
                si = inst.sync_info
                if si is None or len(si.on_wait) <= 1:
                    continue
                tn = type(inst).__name__
                eng = str(inst.engine)
                # merge duplicate-sem waits to the max value first
                best = {}
                for x in si.on_wait:
                    if x.ant_name not in best or x.wait_value > best[x.ant_name].wait_value:
                        best[x.ant_name] = x
                w = list(best.values())
                if len(w) < len(si.on_wait):
                    inst.sync_info = mybir.SyncInfo(
                        on_wait=w, on_update=list(si.on_update)
                    )
                    si = inst.sync_info
                if len(w) <= 1:
                    continue
                if tn in ("InstMatmult", "InstLdweights"):
                    w2 = [x for x in w if not x.ant_name.startswith("PE_")]
                    if len(w2) < len(w) and len(w2) <= 1:
                        inst.sync_info = mybir.SyncInfo(
                            on_wait=w2, on_update=list(si.on_update)
                        )
                elif len(w) > 1 and tn == "InstDrain":
                    # kernel-tail drain: keep the out-DMA wait; NEFF-level
                    # execution barriers cover the rest
                    w.sort(key=lambda x: 0 if x.ant_name.startswith("DMA") else 1)
                    inst.sync_info = mybir.SyncInfo(
                        on_wait=w[:1], on_update=list(si.on_update)
                    )
                elif len(w) > 1 and tn not in ("InstDMACopy",) and not eng.endswith("SP"):
                    # compute instruction. Sound drops for this kernel:
                    # - DMA waits: released-zone bounding-box artifacts
                    # - own-engine sem: engines execute in issue order
                    # - PE waits on Pool ops / Pool waits on DVE ops: no
                    #   such real data deps exist here (zone artifacts)
                    own = {"Pool": "Pool_", "DVE": "DVE_", "Activation": "Activation_"}.get(
                        eng.split(".")[-1], "zz"
                    )
                    w2 = [
                        x
                        for x in w
                        if not (
                            x.ant_name.startswith("DMASW")
                            or x.ant_name.startswith("DMAHW")
                            or x.ant_name.startswith(own)
                            or (eng.endswith("Pool") and x.ant_name.startswith("PE_"))
                            or (eng.endswith("DVE") and x.ant_name.startswith("Pool_")
                                and tn == "InstTensorCopy")
                        )
                    ]
                    if len(w2) > 1:
                        # last resort: keep the most-binding wait
                        rank = {"PE": 0, "Ac": 1, "DV": 2, "Po": 3}
                        w2.sort(key=lambda x: rank.get(x.ant_name[:2], 4))
                        w2 = w2[:1]
                    if not w2:
                        w2 = w[:1]
                    if len(w2) < len(w):
                        inst.sync_info = mybir.SyncInfo(
                            on_wait=w2, on_update=list(si.on_update)
                        )
                elif False:
                    # compute instruction (ACT/DVE/Pool): DMA waits here are
                    # bounding-box artifacts vs long-completed const loads
                    w2 = [
                        x
                        for x in w
                        if not (
                            x.ant_name.startswith("DMASW")
                            or x.ant_name.startswith("DMAHW")
                        )
                    ]
                    if len(w2) < len(w) and len(w2) <= 1:
                        inst.sync_info = mybir.SyncInfo(
                            on_wait=w2, on_update=list(si.on_update)
                        )
                elif tn == "InstDMACopy":
                    # DMA-vs-DMA waits here come from bounding-box overlap
                    # of disjoint scatter regions (out-DMAs) or same-FIFO
                    # SWDGE ordering -- physically redundant either way.
                    w2 = [
                        x
                        for x in w
                        if not (
                            x.ant_name.startswith("DMASW")
                            or x.ant_name.startswith("DMAHW")
                        )
                    ]
                    if len(w2) < len(w) and len(w2) <= 1:
                        inst.sync_info = mybir.SyncInfo(
                            on_wait=w2, on_update=list(si.on_update)
                        )


def host_inputs(X, kernel, chain_kernel, bias, left_boundary, right_boundary):
    """Host-side prep: per-core input maps."""
    X = np.ascontiguousarray(np.asarray(X, np.float32))
    W = np.ascontiguousarray(np.asarray(kernel, np.float32))
    C = np.asarray(chain_kernel, np.float32)
    bias = np.asarray(bias, np.float32)
    lb = np.asarray(left_boundary, np.float32)
    rb = np.asarray(right_boundary, np.float32)

    EW = np.exp(-C.astype(np.float64) - CSCALE).astype(np.float32)  # (F,F)
    NB = np.stack(
        [-(bias + lb), -bias, -(bias + rb), np.zeros_like(bias)], axis=1
    ).astype(np.float32)  # (F,4)
    IDN = np.eye(F, dtype=np.float32)

    in_maps = []
    for c in range(NCORES):
        in_maps.append(
            {
                "x": np.ascontiguousarray(X[c * BL : (c + 1) * BL]),
                "w": W,
                "ew": EW,
                "nb": NB,
                "idn": IDN,
            }
        )
    return in_maps


_NC_CACHE = None


def kernel(X, kernel, chain_kernel, bias, left_boundary, right_boundary):
    global _NC_CACHE
    from concourse.bass_utils import run_bass_kernel_spmd

    if _NC_CACHE is None:
        _NC_CACHE = build_nc()
    nc = _NC_CACHE
    in_maps = host_inputs(X, kernel, chain_kernel, bias, left_boundary, right_boundary)
    res = run_bass_kernel_spmd(nc, in_maps, list(range(NCORES)))
    out = np.concatenate([res.results[c]["out"] for c in range(NCORES)], axis=0)
    return out.astype(np.float32)



# revision 5
# speedup vs baseline: 1.2215x; 1.2215x over previous
"""CRF marginal kernel for Trainium2 (8 NeuronCores, SPMD data-parallel over batch).

Reference math (keras_contrib CRF get_marginal_prob):
  e = X @ W + bias  (+ left/right boundary at t=0 / t=T-1)
  alpha/beta: logsumexp scans over T with transition chain[i,j]
  out = softmax_j(-(alpha_sr + e + beta_sl))

Kernel algorithm (per core, B_local=8), fp16 datapath:
  Linear-domain recurrence with constant per-step rescale c folded into the
  transition weights E'[i,j] = exp(-chain[i,j] - c):
      v_{t+1} = E'^T (v_t * Q_t),   Q_t = exp(-e_t),  v_0 = 1   (fwd)
  and the mirrored bwd scan. Per-(b,t) scale factors cancel in the final
  softmax, so each scan is split into H=8 segments run CONCURRENTLY, each
  burned in BURN=8 steps from an arbitrary init (the transition matrices are
  strongly mixing; fp16 noise floor ~2e-3 rel dominates burn error). Serial
  chain 72 steps. Each step is one [128,128] fp16 tile: 2 dirs x 8 segs x
  8 batch; DVE multiply + PE fp16 matmul on the critical path.

  Final combine, from stored per-step q = state*Q and a bf16 Q^3 buffer
  (q2=Q*Q, q3=q2*Q on DVE; bf16 holds the 4e6 range):
      u[j,(b,t)] = Q^3 / (qf * qb), out = u / sum_j u via PE transpose +
  ACT copy-with-accum + fast-reciprocal + ACT per-partition scale.

  Energy matmul: X pre-cast to fp16 on host (halves HBM traffic), loaded in
  time-stripe order matching recurrence consumption, X^T on-chip via PE fp16
  transposes (fp16 PSUM), fp16 matmuls (N=256), exp fused into PSUM->SBUF
  eviction on ACT with bias/boundary folded in. PSUM->SBUF transpose
  evictions split ACT/DVE to keep ACT under the DMA cadence.
"""

import numpy as np

B, T, D, F = 64, 512, 2048, 128
NCORES = 8
BL = B // NCORES  # 8 batch per core
H = 8  # segments per scan direction
SEG = T // H  # 64
BURN = 8  # burn-in steps per segment (fp16 noise floor; see numpy sim)
NSTEP = SEG + BURN  # 72 tile-steps; muls k=0..71, matmuls k=0..70
NSC = 16  # phase-1 super-chunks (4 time-stripes each)
PAD = BURN * BL  # 64 pad cols each side of QBUF
CSCALE = 5.3513  # mean per-step log-drift (concentration-stable statistic)


def build_nc():
    import concourse.bass as bass
    import concourse.mybir as mybir
    from concourse.tile import TileContext
    from concourse.ap import AP

    fp32 = mybir.dt.float32
    fp16 = mybir.dt.float16
    bf16 = mybir.dt.bfloat16
    Act = mybir.ActivationFunctionType
    Alu = mybir.AluOpType

    nc = bass.Bass()
    Xd = nc.declare_dram_parameter("x", [BL, T, D], fp16, isOutput=False)
    Wd = nc.declare_dram_parameter("w", [D, F], fp16, isOutput=False)
    EWd = nc.declare_dram_parameter("ew", [F, F], fp16, isOutput=False)
    NBd = nc.declare_dram_parameter("nb", [F, 4], fp32, isOutput=False)
    IDd = nc.declare_dram_parameter("idn", [F, F], fp16, isOutput=False)
    OUTd = nc.declare_dram_parameter("out", [BL, T, F], fp32, isOutput=True)

    def sub(base, col_off, dims):
        """Custom free-dim AP into a [128, N] SBUF/PSUM tile view."""
        return AP(
            tensor=base.tensor,
            offset=base.offset + col_off,
            ap=[list(base.ap[0])] + [list(d) for d in dims],
        )

    def qcol(t):  # QBUF column of (t, b=0)
        return PAD + t * BL

    def pump(ap):
        """PE observation pump: a 1-col ldweights with a genuine cross-
        engine data dep. PE matmuls have a single sync-wait slot in
        walrus codegen; this absorbs one producer's wait so the real
        matmul that follows carries at most one."""
        if ap.dtype != fp16:
            ap = ap.bitcast(fp16)
        nc.tensor.ldweights(ap)

    with TileContext(nc) as tc:
        with (
            tc.tile_pool(name="const", bufs=1) as constp,
            tc.tile_pool(name="big", bufs=1) as bigp,
            tc.tile_pool(name="state", bufs=4, space="PSUM") as statep,
        ):
            # ---- constants (issued before any X traffic) ----
            # ew/id go through DVE copies so PE consumers coalesce their
            # wait with other DVE deps (PE matmuls have ONE sync-wait slot).
            ew_ld = constp.tile([128, 128], fp16, name="ew_ld")
            nc.sync.dma_start(out=ew_ld[:], in_=EWd[:])
            ew_sb = constp.tile([128, 128], fp16, name="ew_sb")
            nc.vector.tensor_copy(ew_sb[:], ew_ld[:])
            id_ld = constp.tile([128, 128], fp16, name="id_ld")
            nc.sync.dma_start(out=id_ld[:], in_=IDd[:])
            id_sb = constp.tile([128, 128], fp16, name="id_sb")
            nc.vector.tensor_copy(id_sb[:], id_ld[:])
            nb_sb = constp.tile([128, 4], fp32, name="nb_sb")
            nc.sync.dma_start(out=nb_sb[:], in_=NBd[:])
            # fp16 W direct from DRAM, chunk-major for the energy matmul
            w16 = constp.tile([128, 16 * 128], fp16, name="w16")
            nc.sync.dma_start(
                out=w16[:].rearrange("p (c j) -> p c j", c=16),
                in_=Wd[:].rearrange("(c p) j -> p c j", p=128),
            )

            # ---- persistent big buffers ----
            # QBUF[:, PAD + t*8 + b] = exp(-e[b,t,:]) fp16; PAD cols of 1.0
            qbuf = bigp.tile([128, 2 * PAD + T * BL], fp16, name="qbuf")
            nc.vector.memset(qbuf[:, :PAD], 1.0)
            nc.vector.memset(qbuf[:, PAD + T * BL :], 1.0)
            # Q3BUF[:, t*8 + b] = Q^3 bf16 (cubed on DVE)
            q3buf = bigp.tile([128, T * BL], bf16, name="q3buf")
            # QSTORE step-k tile at cols [k*128, (k+1)*128):
            #   col k*128 + g*8 + b         = fwd seg g
            #   col k*128 + 64 + g*8 + b    = bwd seg g
            qstore = bigp.tile([128, NSTEP * 128], fp16, name="qstore")
            # combine output staging: block bi at cols bi*128 (no reuse ->
            # no WAR between DVE writes and out-DMA reads); fp16, the
            # out-DMA casts to fp32
            obuf = bigp.tile([128, (T // 16) * 128], fp16, name="obuf")
            q2scr = bigp.tile([128, 128], fp16, name="q2scr")
            scrq = bigp.tile([128, NSTEP * 16], fp32, name="scrq")
            scrs = bigp.tile([128, NSTEP], fp32, name="scrs")

            prev_ps = None

            def emit_step(k):
                nonlocal prev_ps
                # fwd seg g at t = g*64 - BURN + k -> col qcol(k-BURN) + g*512
                # bwd seg g at t = (SEG+BURN-1-k) + g*64
                offF = qcol(k - BURN)
                offB = qcol(SEG + BURN - 1 - k)
                qin = sub(
                    qbuf, offF, [[offB - offF, 2], [SEG * BL, H], [1, BL]]
                )
                qout = sub(qstore, k * 128, [[64, 2], [8, H], [1, BL]])
                # DVE pump: sample one col of every Q block the mul reads so
                # the single coalesced ACT wait lands here, not on the mul
                qsamp = sub(qbuf, offF, [[offB - offF, 2], [SEG * BL, H], [1, 1]])
                nc.vector.tensor_copy(
                    sub(scrq, k * 16, [[8, 2], [1, H], [1, 1]]), qsamp
                )
                if k == 0:
                    nc.vector.tensor_copy(qout, qin)
                else:
                    # DVE pump: absorb the PSUM-state (PE) wait so the mul
                    # carries only the single coalesced ACT wait for QBUF
                    nc.vector.tensor_copy(scrs[:, k : k + 1], sub(prev_ps, 0, [[1, 1]]))
                    pin = sub(prev_ps, 0, [[64, 2], [8, H], [1, BL]])
                    nc.vector.tensor_tensor(qout, pin, qin, op=Alu.mult)
                if k == BURN:
                    # exact init: fwd seg0 q = Q_{t=0}, bwd seg7 q = Q_{T-1}
                    ow_out = sub(qstore, k * 128, [[120, 2], [1, BL]])
                    ow_in = sub(qbuf, qcol(0), [[qcol(T - 1) - qcol(0), 2], [1, BL]])
                    nc.vector.tensor_copy(ow_out, ow_in)
                if k < NSTEP - 1:
                    ps = statep.tile([128, 128], fp32, name="st")
                    pump(sub(qstore, k * 128, [[1, 2]]))
                    nc.tensor.matmul(
                        ps[:],
                        ew_sb[:],
                        qstore[:, k * 128 : (k + 1) * 128],
                        start=True,
                        stop=True,
                    )
                    prev_ps = ps

            # ---------------- phase 1 (+ steps it unblocks) ----------------
            with (
                tc.tile_pool(name="xrow", bufs=4) as xrowp,
                tc.tile_pool(name="xtp", bufs=3) as xtp,
                tc.tile_pool(name="ptp", bufs=2, space="PSUM") as ptp,
                tc.tile_pool(name="pep", bufs=2, space="PSUM") as pep,
            ):
                # PE warmup: absorb the id_sb DVE-copy dependency into one
                # throwaway transpose so real transposes only wait the X DMA.
                warm = ptp.tile([128, 512], fp16, name="pt")
                nc.tensor.transpose(warm[:, 0:128], id_sb[:], id_sb[:])
                for s in range(NSC):
                    # stripe production order matched to step consumption
                    # (BURN=8): chunks 0-3 make stripes {56..63, 0..7}
                    # (burn + steps 8..15), chunks 4-15 make stripes
                    # {8..55} (steps 16..39 progressively).
                    if s < 4:
                        rpairs = ((56 + 2 * s, 57 + 2 * s), (6 - 2 * s, 7 - 2 * s))
                    else:
                        m4 = s - 4
                        rpairs = ((8 + 2 * m4, 9 + 2 * m4), (54 - 2 * m4, 55 - 2 * m4))
                    xt = xtp.tile([128, 16 * 256], fp16, name="xt")
                    xrows = []
                    for wi, (r0, _r1) in enumerate(rpairs):
                        xrow = xrowp.tile([128, D], fp16, name="xrow")
                        # rows (b, m, t2): t = r0 + t2 + 64*m
                        xin = AP(
                            tensor=Xd,
                            offset=r0 * D,
                            ap=[[T * D, BL], [SEG * D, 8], [D, 2], [1, D]],
                        )
                        # SWDGE (gpsimd) keeps the whole load on ONE sem so
                        # the consuming PE transposes carry a single wait
                        nc.gpsimd.dma_start(out=xrow[:], in_=xin)
                        xrows.append(xrow)
                    pe = pep.tile([128, 256], fp32, name="pe")

                    def tgroup(wi, dq, gi):
                        # 4 fp16 transposes into one PSUM bank + one wide
                        # PSUM->SBUF copy, alternating ACT/DVE by group idx
                        pump(xrows[wi][:, dq * 512 : dq * 512 + 2])
                        pt = ptp.tile([128, 512], fp16, name="pt")
                        for q in range(4):
                            d = dq * 4 + q
                            nc.tensor.transpose(
                                pt[:, q * 128 : (q + 1) * 128],
                                xrows[wi][:, d * 128 : (d + 1) * 128],
                                id_sb[:],
                            )
                        dst = sub(
                            xt, (dq * 4) * 256 + wi * 128, [[256, 4], [1, 128]]
                        )
                        src = pt[:].rearrange("p (a b) -> p a b", a=4)
                        if gi in (0, 3, 6):
                            nc.scalar.copy(dst, src)
                        else:
                            nc.vector.tensor_copy(dst, src)

                    def mmq(dq):
                        # energy matmuls for d in [4dq, 4dq+4); two pumps
                        # absorb the ACT-half and DVE-half xt-copy waits
                        pump(sub(xt, dq * 4 * 256, [[1, 2]]))
                        pump(sub(xt, dq * 4 * 256 + 128, [[1, 2]]))
                        for d in range(4 * dq, 4 * dq + 4):
                            nc.tensor.matmul(
                                pe[:],
                                w16[:, d * 128 : (d + 1) * 128],
                                xt[:, d * 256 : (d + 1) * 256],
                                start=(d == 0),
                                stop=(d == 15),
                            )

                    tgroup(0, 0, 0)
                    tgroup(0, 1, 1)
                    tgroup(1, 0, 2)
                    mmq(0)
                    tgroup(0, 2, 3)
                    tgroup(1, 1, 4)
                    mmq(1)
                    tgroup(0, 3, 5)
                    tgroup(1, 2, 6)
                    mmq(2)
                    tgroup(1, 3, 7)
                    mmq(3)
                    # fused exp: PSUM -> QBUF (fp16) and -> Q3BUF (bf16).
                    # psum col = wi*128 + b*16 + m*2 + t2
                    for wi, (r0, r1) in enumerate(rpairs):
                        # (bias_col, m0, nm, t2_0, nt2)
                        segs = [(1, 0, 8, 0, 2)]
                        if s == 3 and wi == 1:  # pair (0,1): t=0 at (m=0,t2=0)
                            segs = [(0, 0, 1, 0, 1), (1, 0, 1, 1, 1), (1, 1, 7, 0, 2)]
                        if s == 3 and wi == 0:  # pair (62,63): t=511 at (m=7,t2=1)
                            segs = [(1, 0, 7, 0, 2), (1, 7, 1, 0, 1), (2, 7, 1, 1, 1)]
                        for bcol, m0, nm, t20, nt2 in segs:
                            pin = sub(
                                pe,
                                wi * 128 + m0 * 2 + t20,
                                [[16, BL], [2, nm], [1, nt2]],
                            )
                            qo = sub(
                                qbuf,
                                qcol(r0 + t20 + SEG * m0),
                                [[1, BL], [SEG * BL, nm], [BL, nt2]],
                            )
                            nc.scalar.activation(
                                qo,
                                pin,
                                Act.Exp,
                                bias=nb_sb[:, bcol : bcol + 1],
                                scale=-1.0,
                            )
                        # Q^3 on DVE (exp(-3e) would double the ACT load):
                        # q2 = Q*Q (fp16, 4x), q3 = q2*Q -> bf16 (range 4e6
                        # needs bf16; fp32 internal precision, no overflow)
                        qreg = sub(
                            qbuf, qcol(r0), [[1, BL], [SEG * BL, 8], [BL, 2]]
                        )
                        q2v = q2scr[:].rearrange("p (b m u) -> p b m u", b=BL, m=8)
                        nc.vector.tensor_tensor(q2v, qreg, qreg, op=Alu.mult)
                        q3o = sub(
                            q3buf, r0 * BL, [[1, BL], [SEG * BL, 8], [BL, 2]]
                        )
                        nc.vector.tensor_tensor(q3o, q2v, qreg, op=Alu.mult)
                    if s < 3:
                        emit_step(2 * s)
                        emit_step(2 * s + 1)
                    elif s == 3:
                        for k in range(6, 16):
                            emit_step(k)
                    else:
                        emit_step(2 * s + 8)
                        emit_step(2 * s + 9)

            # ---------------- rest of recurrence + combine ----------------
            with (
                tc.tile_pool(name="comb", bufs=3) as combp,
                tc.tile_pool(name="pup", bufs=2, space="PSUM") as pup,
            ):

                def act_recip(out_ap, in_ap):
                    """Raw ACT Reciprocal (bass wrapper bans it; measured
                    rel err ~2e-6 over [1e-4, 8e3] on HW -- fine at our
                    2e-2 output tolerance)."""
                    imm = lambda v: mybir.ImmediateValue(dtype=fp32, value=v)
                    nc.scalar.add_instruction(
                        mybir.InstActivation(
                            name=nc.get_next_instruction_name(),
                            ins=[nc.scalar.lower_ap(in_ap), imm(0.0), imm(1.0), imm(0.0)],
                            outs=[nc.scalar.lower_ap(out_ap)],
                            func=Act.Reciprocal,
                        )
                    )

                def emit_combine(blks):
                    # 1-2 blocks of 16 t's each, processed with 256-col-wide
                    # elementwise ops to amortize fixed costs. Cols within a
                    # block ordered (b, dt) so the transposed output rows
                    # give contiguous per-batch runs for the out-DMA.
                    nb = len(blks)
                    W = nb * 128

                    def gather(base_fn, dt_stride):
                        offs = [base_fn(t0) for t0 in blks]
                        d0 = (offs[1] - offs[0]) if nb == 2 else 0
                        return offs[0], [[d0, nb], [1, BL], [dt_stride, 16]]

                    o_f, ap_f = gather(
                        lambda t0: (t0 % SEG + BURN) * 128 + (t0 // SEG) * 8, 128
                    )
                    o_b, ap_b = gather(
                        lambda t0: (SEG + BURN - 1 - t0 % SEG) * 128
                        + 64
                        + (t0 // SEG) * 8,
                        -128,
                    )
                    o_3, ap_3 = gather(lambda t0: t0 * BL, BL)
                    qf = sub(qstore, o_f, ap_f)
                    qb = sub(qstore, o_b, ap_b)
                    q3 = sub(q3buf, o_3, ap_3)
                    mb = combp.tile([128, 256], fp32, name="mb")
                    mbv = mb[:, :W].rearrange("p (c b a) -> p c b a", c=nb, b=BL)
                    nc.vector.tensor_tensor(mbv, qf, qb, op=Alu.mult)
                    rm = combp.tile([128, 256], fp32, name="rm")
                    act_recip(rm[:, :W], mb[:, :W])
                    ub = combp.tile([128, 256], fp16, name="ub")
                    nc.vector.tensor_tensor(
                        ub[:, :W].rearrange("p (c b a) -> p c b a", c=nb, b=BL),
                        q3,
                        rm[:, :W].rearrange("p (c b a) -> p c b a", c=nb, b=BL),
                        op=Alu.mult,
                    )
                    pu = pup.tile([128, 256], fp16, name="pu")
                    pump(ub[:, 0:2])
                    for c in range(nb):
                        nc.tensor.transpose(
                            pu[:, c * 128 : (c + 1) * 128],
                            ub[:, c * 128 : (c + 1) * 128],
                            id_sb[:],
                        )
                    ut = combp.tile([128, 256], fp16, name="ut")
                    sm = combp.tile([128, 2], fp32, name="sm")
                    for c in range(nb):
                        nc.scalar.activation(
                            ut[:, c * 128 : (c + 1) * 128],
                            pu[:, c * 128 : (c + 1) * 128],
                            Act.Copy,
                            accum_out=sm[:, c : c + 1],
                        )
                    rs = combp.tile([128, 2], fp32, name="rs")
                    nc.vector.reciprocal(rs[:, :nb], sm[:, :nb])
                    for c, t0 in enumerate(blks):
                        bi = t0 // 16
                        ob = obuf[:, bi * 128 : bi * 128 + 128]
                        nc.vector.tensor_scalar_mul(
                            ob, ut[:, c * 128 : (c + 1) * 128], rs[:, c : c + 1]
                        )
                        # rows b*16+dt -> OUT[b, t0+dt, :]; SWDGE casts
                        # fp16 staging -> fp32 DRAM on the way out
                        oap = AP(
                            tensor=OUTd,
                            offset=t0 * F,
                            ap=[[T * F, BL], [F, 16], [1, F]],
                        )
                        nc.gpsimd.dma_start(out=oap, in_=ob)

                # block t0 ready after step max((t0%64)+BURN+15, ((T-1-t0)%64)+BURN)
                ready = {}
                for blk in range(T // 16):
                    t0 = blk * 16
                    kf = (t0 % SEG) + BURN + 15
                    kb = ((T - 1 - t0) % SEG) + BURN
                    ready.setdefault(max(kf, kb), []).append(t0)

                pending = []
                for k in range(40, NSTEP):
                    emit_step(k)
                    pending.extend(ready.get(k, []))
                    if pending:
                        emit_combine(pending[:2])
                        del pending[:2]
                while pending:
                    emit_combine(pending[:2])
                    del pending[:2]

    _strip_redundant_waits(nc)
    return nc


def _strip_redundant_waits(nc):
    """Drop sync waits that hardware ordering already guarantees, to fit
    walrus's one-sync-wait-per-instruction limit on PE/DMA instructions:
    - PE->PE PSUM WAW waits: PE completions are pc-monotone (documented:
      a single then_inc on the last of concurrent MMs is sound), so an
      earlier PE write always lands before a later one.
    - SWDGE->SWDGE DMA WAW waits: mainline gpsimd DMAs share one physical
      FIFO queue (qPoolDynamic), so they complete in issue order.
    """
    import concourse.mybir as mybir

    for f in nc.m.functions:
        for bb in f.blocks:
            for inst in bb.instructions:
                si = inst.sync_info
                if si is None or len(si.on_wait) <= 1:
                    continue
                tn = type(inst).__name__
                eng = str(inst.engine)
                # merge duplicate-sem waits to the max value first
                best = {}
                for x in si.on_wait:
                    if x.ant_name not in best or x.wait_value > best[x.ant_name].wait_value:
                        best[x.ant_name] = x
                w = list(best.values())
                if len(w) < len(si.on_wait):
                    inst.sync_info = mybir.SyncInfo(
                        on_wait=w, on_update=list(si.on_update)
                    )
                    si = inst.sync_info
                if len(w) <= 1:
                    continue
                if tn in ("InstMatmult", "InstLdweights"):
                    w2 = [x for x in w if not x.ant_name.startswith("PE_")]
                    if len(w2) < len(w) and len(w2) <= 1:
                        inst.sync_info = mybir.SyncInfo(
                            on_wait=w2, on_update=list(si.on_update)
                        )
                elif len(w) > 1 and tn == "InstDrain":
                    # kernel-tail drain: keep the out-DMA wait; NEFF-level
                    # execution barriers cover the rest
                    w.sort(key=lambda x: 0 if x.ant_name.startswith("DMA") else 1)
                    inst.sync_info = mybir.SyncInfo(
                        on_wait=w[:1], on_update=list(si.on_update)
                    )
                elif len(w) > 1 and tn not in ("InstDMACopy",) and not eng.endswith("SP"):
                    # compute instruction. Sound drops for this kernel:
                    # - DMA waits: released-zone bounding-box artifacts
                    # - own-engine sem: engines execute in issue order
                    # - PE waits on Pool ops / Pool waits on DVE ops: no
                    #   such real data deps exist here (zone artifacts)
                    own = {"Pool": "Pool_", "DVE": "DVE_", "Activation": "Activation_"}.get(
                        eng.split(".")[-1], "zz"
                    )
                    w2 = [
                        x
                        for x in w
                        if not (
                            x.ant_name.startswith("DMASW")
                            or x.ant_name.startswith("DMAHW")
                            or x.ant_name.startswith(own)
                            or (eng.endswith("Pool") and x.ant_name.startswith("PE_"))
                            or (eng.endswith("DVE") and x.ant_name.startswith("Pool_")
                                and tn == "InstTensorCopy")
                        )
                    ]
                    if len(w2) > 1:
                        # last resort: keep the most-binding wait
                        rank = {"PE": 0, "Ac": 1, "DV": 2, "Po": 3}
                        w2.sort(key=lambda x: rank.get(x.ant_name[:2], 4))
                        w2 = w2[:1]
                    if not w2:
                        w2 = w[:1]
                    if len(w2) < len(w):
                        inst.sync_info = mybir.SyncInfo(
                            on_wait=w2, on_update=list(si.on_update)
                        )
                elif tn == "InstDMACopy":
                    # DMA-vs-DMA waits here come from bounding-box overlap
                    # of disjoint scatter regions (out-DMAs) or same-FIFO
                    # SWDGE ordering -- physically redundant either way.
                    w2 = [
                        x
                        for x in w
                        if not (
                            x.ant_name.startswith("DMASW")
                            or x.ant_name.startswith("DMAHW")
                        )
                    ]
                    if len(w2) < len(w) and len(w2) <= 1:
                        inst.sync_info = mybir.SyncInfo(
                            on_wait=w2, on_update=list(si.on_update)
                        )


def host_inputs(X, kernel, chain_kernel, bias, left_boundary, right_boundary):
    """Host-side prep: per-core input maps (X/W/EW/ID pre-cast to fp16)."""
    X16 = np.ascontiguousarray(np.asarray(X, np.float16))
    W16 = np.ascontiguousarray(np.asarray(kernel, np.float16))
    C = np.asarray(chain_kernel, np.float32)
    bias = np.asarray(bias, np.float32)
    lb = np.asarray(left_boundary, np.float32)
    rb = np.asarray(right_boundary, np.float32)

    EW = np.exp(-C.astype(np.float64) - CSCALE).astype(np.float16)  # (F,F)
    NB = np.stack(
        [-(bias + lb), -bias, -(bias + rb), np.zeros_like(bias)], axis=1
    ).astype(np.float32)  # (F,4)
    IDN = np.eye(F, dtype=np.float16)

    in_maps = []
    for c in range(NCORES):
        in_maps.append(
            {
                "x": np.ascontiguousarray(X16[c * BL : (c + 1) * BL]),
                "w": W16,
                "ew": EW,
                "nb": NB,
                "idn": IDN,
            }
        )
    return in_maps


_NC_CACHE = None


def kernel(X, kernel, chain_kernel, bias, left_boundary, right_boundary):
    global _NC_CACHE
    from concourse.bass_utils import run_bass_kernel_spmd

    if _NC_CACHE is None:
        _NC_CACHE = build_nc()
    nc = _NC_CACHE
    in_maps = host_inputs(X, kernel, chain_kernel, bias, left_boundary, right_boundary)
    res = run_bass_kernel_spmd(nc, in_maps, list(range(NCORES)))
    out = np.concatenate([res.results[c]["out"] for c in range(NCORES)], axis=0)
    return out.astype(np.float32)


# revision 12
# speedup vs baseline: 1.3684x; 1.1203x over previous
"""CRF marginal kernel for Trainium2 (8 NeuronCores, SPMD data-parallel over batch).

Reference math (keras_contrib CRF get_marginal_prob):
  e = X @ W + bias  (+ left/right boundary at t=0 / t=T-1)
  alpha/beta: logsumexp scans over T with transition chain[i,j]
  out = softmax_j(-(alpha_sr + e + beta_sl))

Kernel algorithm (per core, B_local=8), fp16 datapath:
  Linear-domain recurrence with constant per-step rescale c folded into the
  transition weights E'[i,j] = exp(-chain[i,j] - c):
      v_{t+1} = E'^T (v_t * Q_t),   Q_t = exp(-e_t),  v_0 = 1   (fwd)
  and the mirrored bwd scan. Per-(b,t) scale factors cancel in the final
  softmax, so each scan is split into H=8 segments run CONCURRENTLY, each
  burned in BURN=8 steps from an arbitrary init (the transition matrices are
  strongly mixing; fp16 noise floor ~2e-3 rel dominates burn error). Serial
  chain 72 steps. Each step is one [128,128] fp16 tile: 2 dirs x 8 segs x
  8 batch; DVE multiply + PE fp16 matmul on the critical path.

  Final combine, from stored per-step q = state*Q and a bf16 Q^3 buffer
  (q2=Q*Q, q3=q2*Q on DVE; bf16 holds the 4e6 range):
      u[j,(b,t)] = Q^3 / (qf * qb), out = u / sum_j u via PE transpose +
  ACT copy-with-accum + fast-reciprocal + ACT per-partition scale.

  Energy matmul: X pre-cast to fp16 on host (halves HBM traffic), loaded in
  time-stripe order matching recurrence consumption, X^T on-chip via PE fp16
  transposes (fp16 PSUM), fp16 matmuls (N=256), exp fused into PSUM->SBUF
  eviction on ACT with bias/boundary folded in. PSUM->SBUF transpose
  evictions split ACT/DVE to keep ACT under the DMA cadence.
"""

import numpy as np

B, T, D, F = 64, 512, 2048, 128
NCORES = 8
BL = B // NCORES  # 8 batch per core
H = 8  # segments per scan direction
SEG = T // H  # 64
BURN = 8  # burn-in steps per segment (fp16 noise floor; see numpy sim)
NSTEP = SEG + BURN  # 72 tile-steps; muls k=0..71, matmuls k=0..70
NSC = 16  # phase-1 super-chunks (4 time-stripes each)
PAD = BURN * BL  # 64 pad cols each side of QBUF
CSCALE = 5.3513  # mean per-step log-drift (concentration-stable statistic)


def stripe_pairs(s):
    """Stripe (t-residue) pairs loaded by super-chunk s, ordered to match
    recurrence consumption with BURN=8: chunks 0-3 cover the burn set
    {56..63, 0..7}; chunks 4-15 cover {8..55} in step order."""
    if s < 4:
        return ((56 + 2 * s, 57 + 2 * s), (6 - 2 * s, 7 - 2 * s))
    m4 = s - 4
    return ((8 + 2 * m4, 9 + 2 * m4), (54 - 2 * m4, 55 - 2 * m4))


def build_nc():
    import concourse.bass as bass
    import concourse.mybir as mybir
    from concourse.tile import TileContext
    from concourse.ap import AP

    fp32 = mybir.dt.float32
    fp16 = mybir.dt.float16
    bf16 = mybir.dt.bfloat16
    Act = mybir.ActivationFunctionType
    Alu = mybir.AluOpType

    nc = bass.Bass()
    Xd = nc.declare_dram_parameter("x", [NSC, 2, 128, D], fp16, isOutput=False)
    Wd = nc.declare_dram_parameter("w", [D, F], fp16, isOutput=False)
    EWd = nc.declare_dram_parameter("ew", [F, F], fp16, isOutput=False)
    NBd = nc.declare_dram_parameter("nb", [F, 4], fp32, isOutput=False)
    IDd = nc.declare_dram_parameter("idn", [F, 2 * F + 1], fp16, isOutput=False)
    OUTd = nc.declare_dram_parameter("out", [BL, T, F], fp32, isOutput=True)

    def sub(base, col_off, dims):
        """Custom free-dim AP into a [128, N] SBUF/PSUM tile view."""
        return AP(
            tensor=base.tensor,
            offset=base.offset + col_off,
            ap=[list(base.ap[0])] + [list(d) for d in dims],
        )

    def qcol(t):  # QBUF column of (t, b=0)
        return PAD + t * BL

    def pump(ap):
        """PE observation pump: a 1-col ldweights with a genuine cross-
        engine data dep. PE matmuls have a single sync-wait slot in
        walrus codegen; this absorbs one producer's wait so the real
        matmul that follows carries at most one."""
        if ap.dtype != fp16:
            ap = ap.bitcast(fp16)
        nc.tensor.ldweights(ap)

    with TileContext(nc) as tc:
        with (
            tc.tile_pool(name="const", bufs=1) as constp,
            tc.tile_pool(name="big", bufs=1) as bigp,
            tc.tile_pool(name="state", bufs=4, space="PSUM") as statep,
        ):
            # ---- constants (issued before any X traffic) ----
            # ew/id go through DVE copies so PE consumers coalesce their
            # wait with other DVE deps (PE matmuls have ONE sync-wait slot).
            ew_ld = constp.tile([128, 128], fp16, name="ew_ld")
            nc.sync.dma_start(out=ew_ld[:], in_=EWd[:])
            ew_sb = constp.tile([128, 128], fp16, name="ew_sb")
            nc.vector.tensor_copy(ew_sb[:], ew_ld[:])
            id_ld = constp.tile([128, 257], fp16, name="id_ld")
            nc.sync.dma_start(out=id_ld[:], in_=IDd[:])
            idsum = constp.tile([128, 257], fp16, name="idsum")
            nc.vector.tensor_copy(idsum[:], id_ld[:])
            id_sb = idsum[:, 0:128]  # true identity (phase-1 transposes)
            idc = idsum[:, 128:257]  # 2^-6 * [I | ones] (combine; the scale
            # guards the fp16 row-sum column against overflow and cancels
            # in the softmax normalize)
            nb_sb = constp.tile([128, 4], fp32, name="nb_sb")
            nc.sync.dma_start(out=nb_sb[:], in_=NBd[:])
            # fp16 W direct from DRAM, chunk-major for the energy matmul
            w16 = constp.tile([128, 16 * 128], fp16, name="w16")
            nc.sync.dma_start(
                out=w16[:].rearrange("p (c j) -> p c j", c=16),
                in_=Wd[:].rearrange("(c p) j -> p c j", p=128),
            )

            # ---- persistent big buffers ----
            # QBUF[:, PAD + t*8 + b] = exp(-e[b,t,:]) fp16; PAD cols of 1.0
            qbuf = bigp.tile([128, 2 * PAD + T * BL], fp16, name="qbuf")
            nc.vector.memset(qbuf[:, :PAD], 1.0)
            nc.vector.memset(qbuf[:, PAD + T * BL :], 1.0)
            # Q3BUF[:, t*8 + b] = Q^3 bf16 (cubed on DVE)
            q3buf = bigp.tile([128, T * BL], bf16, name="q3buf")
            # QSTORE step-k tile at cols [k*128, (k+1)*128):
            #   col k*128 + g*8 + b         = fwd seg g
            #   col k*128 + 64 + g*8 + b    = bwd seg g
            qstore = bigp.tile([128, NSTEP * 128], fp16, name="qstore")
            # combine output staging: block bi at cols bi*128 (no reuse ->
            # no WAR between ACT writes and out-DMA reads)
            obuf = bigp.tile([128, (T // 16) * 128], fp32, name="obuf")
            q2scr = bigp.tile([128, 128], fp16, name="q2scr")
            scrq = bigp.tile([128, NSTEP * 16], fp16, name="scrq")
            scrs = bigp.tile([128, NSTEP], fp32, name="scrs")

            prev_ps = None

            def emit_step(k):
                nonlocal prev_ps
                # fwd seg g at t = g*64 - BURN + k -> col qcol(k-BURN) + g*512
                # bwd seg g at t = (SEG+BURN-1-k) + g*64
                offF = qcol(k - BURN)
                offB = qcol(SEG + BURN - 1 - k)
                qin = sub(
                    qbuf, offF, [[offB - offF, 2], [SEG * BL, H], [1, BL]]
                )
                qout = sub(qstore, k * 128, [[64, 2], [8, H], [1, BL]])
                # DVE pump: sample one col of every Q block the mul reads so
                # the single coalesced ACT wait lands here, not on the mul
                qsamp = sub(qbuf, offF, [[offB - offF, 2], [SEG * BL, H], [1, 1]])
                nc.vector.tensor_copy(
                    sub(scrq, k * 16, [[8, 2], [1, H], [1, 1]]), qsamp
                )
                if k == 0:
                    nc.vector.tensor_copy(qout, qin)
                else:
                    # DVE pump: absorb the PSUM-state (PE) wait so the mul
                    # carries only the single coalesced ACT wait for QBUF
                    nc.vector.tensor_copy(scrs[:, k : k + 1], sub(prev_ps, 0, [[1, 1]]))
                    pin = sub(prev_ps, 0, [[64, 2], [8, H], [1, BL]])
                    nc.vector.tensor_tensor(qout, pin, qin, op=Alu.mult)
                if k == BURN:
                    # exact init: fwd seg0 q = Q_{t=0}, bwd seg7 q = Q_{T-1}
                    ow_out = sub(qstore, k * 128, [[120, 2], [1, BL]])
                    ow_in = sub(qbuf, qcol(0), [[qcol(T - 1) - qcol(0), 2], [1, BL]])
                    nc.vector.tensor_copy(ow_out, ow_in)
                if k < NSTEP - 1:
                    ps = statep.tile([128, 128], fp32, name="st")
                    pump(sub(qstore, k * 128, [[1, 2]]))
                    nc.tensor.matmul(
                        ps[:],
                        ew_sb[:],
                        qstore[:, k * 128 : (k + 1) * 128],
                        start=True,
                        stop=True,
                    )
                    prev_ps = ps

            # ---------------- phase 1 (+ steps it unblocks) ----------------
            with (
                tc.tile_pool(name="xrow", bufs=4) as xrowp,
                tc.tile_pool(name="xtp", bufs=3) as xtp,
                tc.tile_pool(name="ptp", bufs=2, space="PSUM") as ptp,
                tc.tile_pool(name="pep", bufs=2, space="PSUM") as pep,
            ):
                # PE warmup: absorb the id_sb DVE-copy dependency into one
                # throwaway transpose so real transposes only wait the X DMA.
                warm = ptp.tile([128, 512], fp16, name="pt")
                nc.tensor.transpose(warm[:, 0:128], id_sb, id_sb)
                for s in range(NSC):
                    rpairs = stripe_pairs(s)
                    xt = xtp.tile([128, 16 * 256], fp16, name="xt")
                    # host pre-gathers stripes: Xd[s][wi][row][d] with
                    # row = b*16 + m*2 + t2, t = r0 + t2 + 64*m -- one
                    # fully-contiguous 1MB DMA per chunk (4KB-scattered
                    # reads measured only 129 GB/s; contiguous ~340)
                    xrow = xrowp.tile([128, 2 * D], fp16, name="xrow")
                    xin = AP(
                        tensor=Xd,
                        offset=s * 2 * 128 * D,
                        ap=[[D, 128], [128 * D, 2], [1, D]],
                    )
                    (nc.gpsimd, nc.sync, nc.scalar)[s % 3].dma_start(
                        out=xrow[:], in_=xin
                    )
                    pe = pep.tile([128, 256], fp32, name="pe")

                    def tgroup(wi, dq, gi):
                        # 4 fp16 transposes into one PSUM bank + one wide
                        # PSUM->SBUF copy, alternating ACT/DVE by group idx
                        pump(xrow[:, wi * D + dq * 512 : wi * D + dq * 512 + 2])
                        pt = ptp.tile([128, 512], fp16, name="pt")
                        for q in range(4):
                            d = dq * 4 + q
                            nc.tensor.transpose(
                                pt[:, q * 128 : (q + 1) * 128],
                                xrow[:, wi * D + d * 128 : wi * D + (d + 1) * 128],
                                id_sb,
                            )
                        dst = sub(
                            xt, (dq * 4) * 256 + wi * 128, [[256, 4], [1, 128]]
                        )
                        src = pt[:].rearrange("p (a b) -> p a b", a=4)
                        if gi in (0, 3, 6):
                            nc.scalar.copy(dst, src)
                        else:
                            nc.vector.tensor_copy(dst, src)

                    def mmq(dq):
                        # energy matmuls for d in [4dq, 4dq+4); two pumps
                        # absorb the ACT-half and DVE-half xt-copy waits
                        pump(sub(xt, dq * 4 * 256, [[1, 2]]))
                        pump(sub(xt, dq * 4 * 256 + 128, [[1, 2]]))
                        for d in range(4 * dq, 4 * dq + 4):
                            nc.tensor.matmul(
                                pe[:],
                                w16[:, d * 128 : (d + 1) * 128],
                                xt[:, d * 256 : (d + 1) * 256],
                                start=(d == 0),
                                stop=(d == 15),
                            )

                    tgroup(0, 0, 0)
                    tgroup(0, 1, 1)
                    tgroup(1, 0, 2)
                    mmq(0)
                    tgroup(0, 2, 3)
                    tgroup(1, 1, 4)
                    mmq(1)
                    tgroup(0, 3, 5)
                    tgroup(1, 2, 6)
                    mmq(2)
                    tgroup(1, 3, 7)
                    mmq(3)
                    # fused exp: PSUM -> QBUF (fp16) and -> Q3BUF (bf16).
                    # psum col = wi*128 + b*16 + m*2 + t2
                    for wi, (r0, r1) in enumerate(rpairs):
                        # (bias_col, m0, nm, t2_0, nt2)
                        segs = [(1, 0, 8, 0, 2)]
                        if s == 3 and wi == 1:  # pair (0,1): t=0 at (m=0,t2=0)
                            segs = [(0, 0, 1, 0, 1), (1, 0, 1, 1, 1), (1, 1, 7, 0, 2)]
                        if s == 3 and wi == 0:  # pair (62,63): t=511 at (m=7,t2=1)
                            segs = [(1, 0, 7, 0, 2), (1, 7, 1, 0, 1), (2, 7, 1, 1, 1)]
                        for bcol, m0, nm, t20, nt2 in segs:
                            pin = sub(
                                pe,
                                wi * 128 + m0 * 2 + t20,
                                [[16, BL], [2, nm], [1, nt2]],
                            )
                            qo = sub(
                                qbuf,
                                qcol(r0 + t20 + SEG * m0),
                                [[1, BL], [SEG * BL, nm], [BL, nt2]],
                            )
                            nc.scalar.activation(
                                qo,
                                pin,
                                Act.Exp,
                                bias=nb_sb[:, bcol : bcol + 1],
                                scale=-1.0,
                            )
                        # Q^3 on DVE (exp(-3e) would double the ACT load):
                        # q2 = Q*Q (fp16, 4x), q3 = q2*Q -> bf16 (range 4e6
                        # needs bf16; fp32 internal precision, no overflow)
                        qreg = sub(
                            qbuf, qcol(r0), [[1, BL], [SEG * BL, 8], [BL, 2]]
                        )
                        q2v = q2scr[:].rearrange("p (b m u) -> p b m u", b=BL, m=8)
                        nc.vector.tensor_tensor(q2v, qreg, qreg, op=Alu.mult)
                        q3o = sub(
                            q3buf, r0 * BL, [[1, BL], [SEG * BL, 8], [BL, 2]]
                        )
                        nc.vector.tensor_tensor(q3o, q2v, qreg, op=Alu.mult)
                    if s < 3:
                        emit_step(2 * s)
                        emit_step(2 * s + 1)
                    elif s == 3:
                        for k in range(6, 16):
                            emit_step(k)
                    else:
                        emit_step(2 * s + 8)
                        emit_step(2 * s + 9)

            # ---------------- rest of recurrence + combine ----------------
            with (
                tc.tile_pool(name="comb", bufs=3) as combp,
                tc.tile_pool(name="pup", bufs=2, space="PSUM") as pup,
            ):

                def act_recip(out_ap, in_ap):
                    """Raw ACT Reciprocal (bass wrapper bans it; measured
                    rel err ~2e-6 over [1e-4, 8e3] on HW -- fine at our
                    2e-2 output tolerance)."""
                    imm = lambda v: mybir.ImmediateValue(dtype=fp32, value=v)
                    nc.scalar.add_instruction(
                        mybir.InstActivation(
                            name=nc.get_next_instruction_name(),
                            ins=[nc.scalar.lower_ap(in_ap), imm(0.0), imm(1.0), imm(0.0)],
                            outs=[nc.scalar.lower_ap(out_ap)],
                            func=Act.Reciprocal,
                        )
                    )

                def emit_combine(blks, alt):
                    # 1-2 blocks of 16 t's; 256-col-wide elementwise ops
                    # amortize fixed costs. Cols within a block ordered
                    # (b, dt) so transposed rows give contiguous per-batch
                    # runs for the out-DMA.
                    nb = len(blks)
                    W = nb * 128

                    def gather(base_fn, dt_stride):
                        offs = [base_fn(t0) for t0 in blks]
                        d0 = (offs[1] - offs[0]) if nb == 2 else 0
                        return offs[0], [[d0, nb], [1, BL], [dt_stride, 16]]

                    o_f, ap_f = gather(
                        lambda t0: (t0 % SEG + BURN) * 128 + (t0 // SEG) * 8, 128
                    )
                    o_b, ap_b = gather(
                        lambda t0: (SEG + BURN - 1 - t0 % SEG) * 128
                        + 64
                        + (t0 // SEG) * 8,
                        -128,
                    )
                    o_3, ap_3 = gather(lambda t0: t0 * BL, BL)
                    qf = sub(qstore, o_f, ap_f)
                    qb = sub(qstore, o_b, ap_b)
                    q3 = sub(q3buf, o_3, ap_3)
                    mb = combp.tile([128, 256], fp32, name="mb")
                    mbv = mb[:, :W].rearrange("p (c b a) -> p c b a", c=nb, b=BL)
                    nc.vector.tensor_tensor(mbv, qf, qb, op=Alu.mult)
                    # bf16 reciprocal keeps u all-2-byte (DVE fast path)
                    rm = combp.tile([128, 256], bf16, name="rm")
                    act_recip(rm[:, :W], mb[:, :W])
                    ub = combp.tile([128, 256], fp16, name="ub")
                    nc.vector.tensor_tensor(
                        ub[:, :W].rearrange("p (c b a) -> p c b a", c=nb, b=BL),
                        q3,
                        rm[:, :W].rearrange("p (c b a) -> p c b a", c=nb, b=BL),
                        op=Alu.mult,
                    )
                    pu = pup.tile([128, 256], fp16, name="pu")
                    pump(ub[:, 0:2])
                    for c in range(nb):
                        nc.tensor.transpose(
                            pu[:, c * 128 : (c + 1) * 128],
                            ub[:, c * 128 : (c + 1) * 128],
                            id_sb,
                        )
                    ut = combp.tile([128, 256], fp16, name="ut")
                    nc.scalar.copy(ut[:, :W], pu[:, :W])
                    sm = combp.tile([128, 2], fp32, name="sm")
                    nc.vector.reduce_sum(
                        sm[:, :nb].rearrange("p (c u) -> p c u", c=nb),
                        ut[:, :W].rearrange("p (c j) -> p c j", c=nb),
                        axis=mybir.AxisListType.X,
                    )
                    rs = combp.tile([128, 2], fp32, name="rs")
                    nc.vector.reciprocal(rs[:, :nb], sm[:, :nb])
                    for c, t0 in enumerate(blks):
                        bi = t0 // 16
                        ob = obuf[:, bi * 128 : bi * 128 + 128]
                        if alt:  # balance the normalize between ACT and DVE
                            nc.scalar.activation(
                                ob,
                                ut[:, c * 128 : (c + 1) * 128],
                                Act.Copy,
                                scale=rs[:, c : c + 1],
                            )
                        else:
                            nc.vector.tensor_scalar_mul(
                                ob, ut[:, c * 128 : (c + 1) * 128], rs[:, c : c + 1]
                            )
                        oap = AP(
                            tensor=OUTd,
                            offset=t0 * F,
                            ap=[[T * F, BL], [F, 16], [1, F]],
                        )
                        nc.sync.dma_start(out=oap, in_=ob)

                # block t0 ready after step max((t0%64)+BURN+15, ((T-1-t0)%64)+BURN)
                ready = {}
                for blk in range(T // 16):
                    t0 = blk * 16
                    kf = (t0 % SEG) + BURN + 15
                    kb = ((T - 1 - t0) % SEG) + BURN
                    ready.setdefault(max(kf, kb), []).append(t0)

                pending = []
                nemit = 0
                for k in range(40, NSTEP):
                    emit_step(k)
                    pending.extend(ready.get(k, []))
                    if pending:
                        emit_combine(pending[:2], nemit % 2 == 0)
                        del pending[:2]
                        nemit += 1
                while pending:
                    emit_combine(pending[:2], nemit % 2 == 0)
                    del pending[:2]
                    nemit += 1

    _strip_redundant_waits(nc)
    return nc


def _strip_redundant_waits(nc):
    """Drop sync waits that hardware ordering already guarantees, to fit
    walrus's one-sync-wait-per-instruction limit on PE/DMA instructions:
    - PE->PE PSUM WAW waits: PE completions are pc-monotone (documented:
      a single then_inc on the last of concurrent MMs is sound), so an
      earlier PE write always lands before a later one.
    - SWDGE->SWDGE DMA WAW waits: mainline gpsimd DMAs share one physical
      FIFO queue (qPoolDynamic), so they complete in issue order.
    """
    import concourse.mybir as mybir

    for f in nc.m.functions:
        for bb in f.blocks:
            for inst in bb.instructions:
                si = inst.sync_info
                if si is None or len(si.on_wait) <= 1:
                    continue
                tn = type(inst).__name__
                eng = str(inst.engine)
                # merge duplicate-sem waits to the max value first
                best = {}
                for x in si.on_wait:
                    if x.ant_name not in best or x.wait_value > best[x.ant_name].wait_value:
                        best[x.ant_name] = x
                w = list(best.values())
                if len(w) < len(si.on_wait):
                    inst.sync_info = mybir.SyncInfo(
                        on_wait=w, on_update=list(si.on_update)
                    )
                    si = inst.sync_info
                if len(w) <= 1:
                    continue
                if tn in ("InstMatmult", "InstLdweights"):
                    w2 = [x for x in w if not x.ant_name.startswith("PE_")]
                    if len(w2) < len(w) and len(w2) <= 1:
                        inst.sync_info = mybir.SyncInfo(
                            on_wait=w2, on_update=list(si.on_update)
                        )
                elif len(w) > 1 and tn == "InstDrain":
                    # kernel-tail drain: keep the out-DMA wait; NEFF-level
                    # execution barriers cover the rest
                    w.sort(key=lambda x: 0 if x.ant_name.startswith("DMA") else 1)
                    inst.sync_info = mybir.SyncInfo(
                        on_wait=w[:1], on_update=list(si.on_update)
                    )
                elif len(w) > 1 and tn not in ("InstDMACopy",) and not eng.endswith("SP"):
                    # compute instruction. Sound drops for this kernel:
                    # - DMA waits: released-zone bounding-box artifacts
                    # - own-engine sem: engines execute in issue order
                    # - PE waits on Pool ops / Pool waits on DVE ops: no
                    #   such real data deps exist here (zone artifacts)
                    own = {"Pool": "Pool_", "DVE": "DVE_", "Activation": "Activation_"}.get(
                        eng.split(".")[-1], "zz"
                    )
                    w2 = [
                        x
                        for x in w
                        if not (
                            x.ant_name.startswith("DMASW")
                            or x.ant_name.startswith("DMAHW")
                            or x.ant_name.startswith(own)
                            or (eng.endswith("Pool") and x.ant_name.startswith("PE_"))
                            or (eng.endswith("DVE") and x.ant_name.startswith("Pool_")
                                and tn == "InstTensorCopy")
                        )
                    ]
                    if len(w2) > 1:
                        # last resort: keep the most-binding wait
                        rank = {"PE": 0, "Ac": 1, "DV": 2, "Po": 3}
                        w2.sort(key=lambda x: rank.get(x.ant_name[:2], 4))
                        w2 = w2[:1]
                    if not w2:
                        w2 = w[:1]
                    if len(w2) < len(w):
                        inst.sync_info = mybir.SyncInfo(
                            on_wait=w2, on_update=list(si.on_update)
                        )
                elif tn == "InstDMACopy":
                    # DMA-vs-DMA waits here come from bounding-box overlap
                    # of disjoint scatter regions (out-DMAs) or same-FIFO
                    # SWDGE ordering -- physically redundant either way.
                    w2 = [
                        x
                        for x in w
                        if not (
                            x.ant_name.startswith("DMASW")
                            or x.ant_name.startswith("DMAHW")
                        )
                    ]
                    if len(w2) < len(w) and len(w2) <= 1:
                        inst.sync_info = mybir.SyncInfo(
                            on_wait=w2, on_update=list(si.on_update)
                        )


def host_inputs(X, kernel, chain_kernel, bias, left_boundary, right_boundary):
    """Host-side prep: per-core input maps (X/W/EW/ID pre-cast to fp16)."""
    X16 = np.asarray(X, np.float16)
    # pre-gather stripes into the kernel's load order: [s][wi][row][d],
    # row = b*16 + m*2 + t2 with t = r0 + t2 + 64*m
    tidx = np.empty((NSC, 2, 8, 2), np.int64)
    for s in range(NSC):
        for wi, (r0, _r1) in enumerate(stripe_pairs(s)):
            for m in range(8):
                for t2 in range(2):
                    tidx[s, wi, m, t2] = r0 + t2 + SEG * m
    W16 = np.ascontiguousarray(np.asarray(kernel, np.float16))
    C = np.asarray(chain_kernel, np.float32)
    bias = np.asarray(bias, np.float32)
    lb = np.asarray(left_boundary, np.float32)
    rb = np.asarray(right_boundary, np.float32)

    EW = np.exp(-C.astype(np.float64) - CSCALE).astype(np.float16)  # (F,F)
    NB = np.stack(
        [-(bias + lb), -bias, -(bias + rb), np.zeros_like(bias)], axis=1
    ).astype(np.float32)  # (F,4)
    IDN = np.concatenate(
        [
            np.eye(F, dtype=np.float32),
            np.eye(F, dtype=np.float32) * 2.0**-6,
            np.ones((F, 1), np.float32) * 2.0**-6,
        ],
        axis=1,
    ).astype(np.float16)

    in_maps = []
    for c in range(NCORES):
        in_maps.append(
            {
                "x": np.ascontiguousarray(
                    X16[c * BL : (c + 1) * BL][:, tidx.reshape(-1), :]
                    .reshape(BL, NSC, 2, 8, 2, D)
                    .transpose(1, 2, 0, 3, 4, 5)
                    .reshape(NSC, 2, 128, D)
                ),
                "w": W16,
                "ew": EW,
                "nb": NB,
                "idn": IDN,
            }
        )
    return in_maps


_NC_CACHE = None


def kernel(X, kernel, chain_kernel, bias, left_boundary, right_boundary):
    global _NC_CACHE
    from concourse.bass_utils import run_bass_kernel_spmd

    if _NC_CACHE is None:
        _NC_CACHE = build_nc()
    nc = _NC_CACHE
    in_maps = host_inputs(X, kernel, chain_kernel, bias, left_boundary, right_boundary)
    res = run_bass_kernel_spmd(nc, in_maps, list(range(NCORES)))
    out = np.concatenate([res.results[c]["out"] for c in range(NCORES)], axis=0)
    return out.astype(np.float32)


# revision 14
# speedup vs baseline: 1.4261x; 1.0422x over previous
"""CRF marginal kernel for Trainium2 (8 NeuronCores, SPMD data-parallel over batch).

Reference math (keras_contrib CRF get_marginal_prob):
  e = X @ W + bias  (+ left/right boundary at t=0 / t=T-1)
  alpha/beta: logsumexp scans over T with transition chain[i,j]
  out = softmax_j(-(alpha_sr + e + beta_sl))

Kernel algorithm (per core, B_local=8), fp16 datapath:
  Linear-domain recurrence with constant per-step rescale c folded into the
  transition weights E'[i,j] = exp(-chain[i,j] - c):
      v_{t+1} = E'^T (v_t * Q_t),   Q_t = exp(-e_t),  v_0 = 1   (fwd)
  and the mirrored bwd scan. Per-(b,t) scale factors cancel in the final
  softmax, so each scan is split into H=8 segments run CONCURRENTLY, each
  burned in BURN=8 steps from an arbitrary init (the transition matrices are
  strongly mixing; fp16 noise floor ~2e-3 rel dominates burn error). Serial
  chain 72 steps. Each step is one [128,128] fp16 tile: 2 dirs x 8 segs x
  8 batch; DVE multiply + PE fp16 matmul on the critical path.

  Final combine, from stored per-step q = state*Q and a bf16 Q^3 buffer
  (q2=Q*Q, q3=q2*Q on DVE; bf16 holds the 4e6 range):
      u[j,(b,t)] = Q^3 / (qf * qb), out = u / sum_j u via PE transpose +
  ACT copy-with-accum + fast-reciprocal + ACT per-partition scale.

  Energy matmul: X pre-cast to fp16 on host (halves HBM traffic), loaded in
  time-stripe order matching recurrence consumption, X^T on-chip via PE fp16
  transposes (fp16 PSUM), fp16 matmuls (N=256), exp fused into PSUM->SBUF
  eviction on ACT with bias/boundary folded in. PSUM->SBUF transpose
  evictions split ACT/DVE to keep ACT under the DMA cadence.
"""

import numpy as np

B, T, D, F = 64, 512, 2048, 128
NCORES = 8
BL = B // NCORES  # 8 batch per core
H = 8  # segments per scan direction
SEG = T // H  # 64
BURN = 8  # burn-in steps per segment (fp16 noise floor; see numpy sim)
NSTEP = SEG + BURN  # 72 tile-steps; muls k=0..71, matmuls k=0..70
NSC = 16  # phase-1 super-chunks (4 time-stripes each)
PAD = BURN * BL  # 64 pad cols each side of QBUF
CSCALE = 5.3513  # mean per-step log-drift (concentration-stable statistic)


def stripe_pairs(s):
    """Stripe (t-residue) pairs loaded by super-chunk s, ordered to match
    recurrence consumption with BURN=8: chunks 0-3 cover the burn set
    {56..63, 0..7}; chunks 4-15 cover {8..55} in step order."""
    if s < 4:
        return ((56 + 2 * s, 57 + 2 * s), (6 - 2 * s, 7 - 2 * s))
    m4 = s - 4
    return ((8 + 2 * m4, 9 + 2 * m4), (54 - 2 * m4, 55 - 2 * m4))


def build_nc():
    import concourse.bass as bass
    import concourse.mybir as mybir
    from concourse.tile import TileContext
    from concourse.ap import AP

    fp32 = mybir.dt.float32
    fp16 = mybir.dt.float16
    bf16 = mybir.dt.bfloat16
    Act = mybir.ActivationFunctionType
    Alu = mybir.AluOpType

    nc = bass.Bass()
    Xd = nc.declare_dram_parameter("x", [NSC, 2, 128, D], fp16, isOutput=False)
    Wd = nc.declare_dram_parameter("w", [D, F], fp16, isOutput=False)
    EWd = nc.declare_dram_parameter("ew", [F, F], fp16, isOutput=False)
    NBd = nc.declare_dram_parameter("nb", [F, 4], fp32, isOutput=False)
    IDd = nc.declare_dram_parameter("idn", [F, 2 * F + 1], fp16, isOutput=False)
    OUTd = nc.declare_dram_parameter("out", [BL, T, F], fp32, isOutput=True)

    def sub(base, col_off, dims):
        """Custom free-dim AP into a [128, N] SBUF/PSUM tile view."""
        return AP(
            tensor=base.tensor,
            offset=base.offset + col_off,
            ap=[list(base.ap[0])] + [list(d) for d in dims],
        )

    def qcol(t):  # QBUF column of (t, b=0)
        return PAD + t * BL

    def pump(ap):
        """PE observation pump: a 1-col ldweights with a genuine cross-
        engine data dep. PE matmuls have a single sync-wait slot in
        walrus codegen; this absorbs one producer's wait so the real
        matmul that follows carries at most one."""
        if ap.dtype != fp16:
            ap = ap.bitcast(fp16)
        nc.tensor.ldweights(ap)

    with TileContext(nc) as tc:
        with (
            tc.tile_pool(name="const", bufs=1) as constp,
            tc.tile_pool(name="big", bufs=1) as bigp,
            tc.tile_pool(name="state", bufs=2, space="PSUM") as statep,
        ):
            # ---- constants (issued before any X traffic) ----
            # ew/id go through DVE copies so PE consumers coalesce their
            # wait with other DVE deps (PE matmuls have ONE sync-wait slot).
            ew_ld = constp.tile([128, 128], fp16, name="ew_ld")
            nc.sync.dma_start(out=ew_ld[:], in_=EWd[:])
            ew_sb = constp.tile([128, 128], fp16, name="ew_sb")
            nc.vector.tensor_copy(ew_sb[:], ew_ld[:])
            id_ld = constp.tile([128, 257], fp16, name="id_ld")
            nc.sync.dma_start(out=id_ld[:], in_=IDd[:])
            idsum = constp.tile([128, 257], fp16, name="idsum")
            nc.vector.tensor_copy(idsum[:], id_ld[:])
            id_sb = idsum[:, 0:128]  # true identity (phase-1 transposes)
            idc = idsum[:, 128:257]  # 2^-6 * [I | ones] (combine; the scale
            # guards the fp16 row-sum column against overflow and cancels
            # in the softmax normalize)
            nb_sb = constp.tile([128, 4], fp32, name="nb_sb")
            nc.sync.dma_start(out=nb_sb[:], in_=NBd[:])
            # fp16 W direct from DRAM, chunk-major for the energy matmul
            w16 = constp.tile([128, 16 * 128], fp16, name="w16")
            nc.sync.dma_start(
                out=w16[:].rearrange("p (c j) -> p c j", c=16),
                in_=Wd[:].rearrange("(c p) j -> p c j", p=128),
            )

            # ---- persistent big buffers ----
            # QBUF[:, PAD + t*8 + b] = exp(-e[b,t,:]) fp16; PAD cols of 1.0
            qbuf = bigp.tile([128, 2 * PAD + T * BL], fp16, name="qbuf")
            nc.vector.memset(qbuf[:, :PAD], 1.0)
            nc.vector.memset(qbuf[:, PAD + T * BL :], 1.0)
            # Q3BUF[:, t*8 + b] = Q^3 bf16 (cubed on DVE)
            q3buf = bigp.tile([128, T * BL], bf16, name="q3buf")
            # QSTORE step-k tile at cols [k*128, (k+1)*128):
            #   col k*128 + g*8 + b         = fwd seg g
            #   col k*128 + 64 + g*8 + b    = bwd seg g
            qstore = bigp.tile([128, NSTEP * 128], fp16, name="qstore")
            # combine output staging: block bi at cols bi*128 (no reuse ->
            # no WAR between ACT writes and out-DMA reads)
            obuf = bigp.tile([128, (T // 16) * 128], fp32, name="obuf")
            q2scr = bigp.tile([128, 128], fp16, name="q2scr")
            scrq = bigp.tile([128, NSTEP * 16], fp16, name="scrq")
            scrs = bigp.tile([128, NSTEP], fp32, name="scrs")

            prev_ps = None

            def emit_step(k):
                nonlocal prev_ps
                # fwd seg g at t = g*64 - BURN + k -> col qcol(k-BURN) + g*512
                # bwd seg g at t = (SEG+BURN-1-k) + g*64
                offF = qcol(k - BURN)
                offB = qcol(SEG + BURN - 1 - k)
                qin = sub(
                    qbuf, offF, [[offB - offF, 2], [SEG * BL, H], [1, BL]]
                )
                qout = sub(qstore, k * 128, [[64, 2], [8, H], [1, BL]])
                # DVE pump: sample one col of every Q block the mul reads so
                # the single coalesced ACT wait lands here, not on the mul
                if k < 40:
                    qsamp = sub(
                        qbuf, offF, [[offB - offF, 2], [SEG * BL, H], [1, 1]]
                    )
                    nc.vector.tensor_copy(
                        sub(scrq, k * 16, [[8, 2], [1, H], [1, 1]]), qsamp
                    )
                if k == 0:
                    nc.vector.tensor_copy(qout, qin)
                else:
                    if k < 40:
                        # DVE pump: absorb the PSUM-state (PE) wait so the
                        # mul carries only the coalesced ACT wait for QBUF
                        nc.vector.tensor_copy(
                            scrs[:, k : k + 1], sub(prev_ps, 0, [[1, 1]])
                        )
                    pin = sub(prev_ps, 0, [[64, 2], [8, H], [1, BL]])
                    nc.vector.tensor_tensor(qout, pin, qin, op=Alu.mult)
                if k == BURN:
                    # exact init: fwd seg0 q = Q_{t=0}, bwd seg7 q = Q_{T-1}
                    ow_out = sub(qstore, k * 128, [[120, 2], [1, BL]])
                    ow_in = sub(qbuf, qcol(0), [[qcol(T - 1) - qcol(0), 2], [1, BL]])
                    nc.vector.tensor_copy(ow_out, ow_in)
                if k < NSTEP - 1:
                    ps = statep.tile([128, 128], fp32, name="st")
                    pump(sub(qstore, k * 128, [[1, 2]]))
                    nc.tensor.matmul(
                        ps[:],
                        ew_sb[:],
                        qstore[:, k * 128 : (k + 1) * 128],
                        start=True,
                        stop=True,
                    )
                    prev_ps = ps

            # ---------------- phase 1 (+ steps it unblocks) ----------------
            with (
                tc.tile_pool(name="xrow", bufs=6) as xrowp,
                tc.tile_pool(name="xtp", bufs=4) as xtp,
                tc.tile_pool(name="ptp", bufs=4, space="PSUM") as ptp,
                tc.tile_pool(name="pep", bufs=2, space="PSUM") as pep,
            ):
                # PE warmup: absorb the id_sb DVE-copy dependency into one
                # throwaway transpose so real transposes only wait the X DMA.
                warm = ptp.tile([128, 512], fp16, name="pt")
                nc.tensor.transpose(warm[:, 0:128], id_sb, id_sb)
                for s in range(NSC):
                    rpairs = stripe_pairs(s)
                    xt = xtp.tile([128, 16 * 256], fp16, name="xt")
                    # host pre-gathers stripes: Xd[s][wi][row][d] with
                    # row = b*16 + m*2 + t2, t = r0 + t2 + 64*m -- one
                    # fully-contiguous 1MB DMA per chunk (4KB-scattered
                    # reads measured only 129 GB/s; contiguous ~340)
                    xrow = xrowp.tile([128, 2 * D], fp16, name="xrow")
                    xin = AP(
                        tensor=Xd,
                        offset=s * 2 * 128 * D,
                        ap=[[D, 128], [128 * D, 2], [1, D]],
                    )
                    (nc.gpsimd, nc.sync, nc.scalar)[s % 3].dma_start(
                        out=xrow[:], in_=xin
                    )
                    pe = pep.tile([128, 256], fp32, name="pe")

                    def tgroup(wi, dq, gi):
                        # 4 fp16 transposes into one PSUM bank + one wide
                        # PSUM->SBUF copy, alternating ACT/DVE by group idx
                        pump(xrow[:, wi * D + dq * 512 : wi * D + dq * 512 + 2])
                        pt = ptp.tile([128, 512], fp16, name="pt")
                        for q in range(4):
                            d = dq * 4 + q
                            nc.tensor.transpose(
                                pt[:, q * 128 : (q + 1) * 128],
                                xrow[:, wi * D + d * 128 : wi * D + (d + 1) * 128],
                                id_sb,
                            )
                        dst = sub(
                            xt, (dq * 4) * 256 + wi * 128, [[256, 4], [1, 128]]
                        )
                        src = pt[:].rearrange("p (a b) -> p a b", a=4)
                        if gi in (0, 3, 6):
                            nc.scalar.copy(dst, src)
                        else:
                            nc.vector.tensor_copy(dst, src)

                    def mmq(dq):
                        # energy matmuls for d in [4dq, 4dq+4); two pumps
                        # absorb the ACT-half and DVE-half xt-copy waits
                        pump(sub(xt, dq * 4 * 256, [[1, 2]]))
                        pump(sub(xt, dq * 4 * 256 + 128, [[1, 2]]))
                        for d in range(4 * dq, 4 * dq + 4):
                            nc.tensor.matmul(
                                pe[:],
                                w16[:, d * 128 : (d + 1) * 128],
                                xt[:, d * 256 : (d + 1) * 256],
                                start=(d == 0),
                                stop=(d == 15),
                            )

                    tgroup(0, 0, 0)
                    tgroup(0, 1, 1)
                    tgroup(1, 0, 2)
                    mmq(0)
                    tgroup(0, 2, 3)
                    tgroup(1, 1, 4)
                    mmq(1)
                    tgroup(0, 3, 5)
                    tgroup(1, 2, 6)
                    mmq(2)
                    tgroup(1, 3, 7)
                    mmq(3)
                    # fused exp: PSUM -> QBUF (fp16) and -> Q3BUF (bf16).
                    # psum col = wi*128 + b*16 + m*2 + t2
                    for wi, (r0, r1) in enumerate(rpairs):
                        # (bias_col, m0, nm, t2_0, nt2)
                        segs = [(1, 0, 8, 0, 2)]
                        if s == 3 and wi == 1:  # pair (0,1): t=0 at (m=0,t2=0)
                            segs = [(0, 0, 1, 0, 1), (1, 0, 1, 1, 1), (1, 1, 7, 0, 2)]
                        if s == 3 and wi == 0:  # pair (62,63): t=511 at (m=7,t2=1)
                            segs = [(1, 0, 7, 0, 2), (1, 7, 1, 0, 1), (2, 7, 1, 1, 1)]
                        for bcol, m0, nm, t20, nt2 in segs:
                            pin = sub(
                                pe,
                                wi * 128 + m0 * 2 + t20,
                                [[16, BL], [2, nm], [1, nt2]],
                            )
                            qo = sub(
                                qbuf,
                                qcol(r0 + t20 + SEG * m0),
                                [[1, BL], [SEG * BL, nm], [BL, nt2]],
                            )
                            nc.scalar.activation(
                                qo,
                                pin,
                                Act.Exp,
                                bias=nb_sb[:, bcol : bcol + 1],
                                scale=-1.0,
                            )
                        # Q^3 on DVE (exp(-3e) would double the ACT load):
                        # q2 = Q*Q (fp16, 4x), q3 = q2*Q -> bf16 (range 4e6
                        # needs bf16; fp32 internal precision, no overflow)
                        qreg = sub(
                            qbuf, qcol(r0), [[1, BL], [SEG * BL, 8], [BL, 2]]
                        )
                        q2v = q2scr[:].rearrange("p (b m u) -> p b m u", b=BL, m=8)
                        nc.vector.tensor_tensor(q2v, qreg, qreg, op=Alu.mult)
                        q3o = sub(
                            q3buf, r0 * BL, [[1, BL], [SEG * BL, 8], [BL, 2]]
                        )
                        nc.vector.tensor_tensor(q3o, q2v, qreg, op=Alu.mult)
                    if s < 3:
                        emit_step(2 * s)
                        emit_step(2 * s + 1)
                    elif s == 3:
                        for k in range(6, 16):
                            emit_step(k)
                    else:
                        emit_step(2 * s + 8)
                        emit_step(2 * s + 9)

            # ---------------- rest of recurrence + combine ----------------
            with (
                tc.tile_pool(name="comb", bufs=3) as combp,
                tc.tile_pool(name="pup", bufs=2, space="PSUM") as pup,
            ):

                def act_recip(out_ap, in_ap):
                    """Raw ACT Reciprocal (bass wrapper bans it; measured
                    rel err ~2e-6 over [1e-4, 8e3] on HW -- fine at our
                    2e-2 output tolerance)."""
                    imm = lambda v: mybir.ImmediateValue(dtype=fp32, value=v)
                    nc.scalar.add_instruction(
                        mybir.InstActivation(
                            name=nc.get_next_instruction_name(),
                            ins=[nc.scalar.lower_ap(in_ap), imm(0.0), imm(1.0), imm(0.0)],
                            outs=[nc.scalar.lower_ap(out_ap)],
                            func=Act.Reciprocal,
                        )
                    )

                def emit_combine(blks):
                    # 1-2 blocks of 16 t's; 256-col-wide elementwise ops
                    # amortize fixed costs. Cols within a block ordered
                    # (b, dt) so transposed rows give contiguous per-batch
                    # runs for the out-DMA.
                    nb = len(blks)
                    W = nb * 128

                    def gather(base_fn, dt_stride):
                        offs = [base_fn(t0) for t0 in blks]
                        d0 = (offs[1] - offs[0]) if nb == 2 else 0
                        return offs[0], [[d0, nb], [1, BL], [dt_stride, 16]]

                    o_f, ap_f = gather(
                        lambda t0: (t0 % SEG + BURN) * 128 + (t0 // SEG) * 8, 128
                    )
                    o_b, ap_b = gather(
                        lambda t0: (SEG + BURN - 1 - t0 % SEG) * 128
                        + 64
                        + (t0 // SEG) * 8,
                        -128,
                    )
                    o_3, ap_3 = gather(lambda t0: t0 * BL, BL)
                    qf = sub(qstore, o_f, ap_f)
                    qb = sub(qstore, o_b, ap_b)
                    q3 = sub(q3buf, o_3, ap_3)
                    mb = combp.tile([128, 256], fp32, name="mb")
                    mbv = mb[:, :W].rearrange("p (c b a) -> p c b a", c=nb, b=BL)
                    nc.vector.tensor_tensor(mbv, qf, qb, op=Alu.mult)
                    # bf16 reciprocal keeps u all-2-byte (DVE fast path)
                    rm = combp.tile([128, 256], bf16, name="rm")
                    act_recip(rm[:, :W], mb[:, :W])
                    ub = combp.tile([128, 256], fp16, name="ub")
                    nc.vector.tensor_tensor(
                        ub[:, :W].rearrange("p (c b a) -> p c b a", c=nb, b=BL),
                        q3,
                        rm[:, :W].rearrange("p (c b a) -> p c b a", c=nb, b=BL),
                        op=Alu.mult,
                    )
                    pu = pup.tile([128, 256], fp16, name="pu")
                    pump(ub[:, 0:2])
                    for c in range(nb):
                        nc.tensor.transpose(
                            pu[:, c * 128 : (c + 1) * 128],
                            ub[:, c * 128 : (c + 1) * 128],
                            id_sb,
                        )
                    ut = combp.tile([128, 256], fp16, name="ut")
                    nc.scalar.copy(ut[:, :W], pu[:, :W])
                    sm = combp.tile([128, 2], fp32, name="sm")
                    nc.vector.reduce_sum(
                        sm[:, :nb].rearrange("p (c u) -> p c u", c=nb),
                        ut[:, :W].rearrange("p (c j) -> p c j", c=nb),
                        axis=mybir.AxisListType.X,
                    )
                    rs = combp.tile([128, 2], fp32, name="rs")
                    nc.vector.reciprocal(rs[:, :nb], sm[:, :nb])
                    for c, t0 in enumerate(blks):
                        bi = t0 // 16
                        ob = obuf[:, bi * 128 : bi * 128 + 128]
                        nc.scalar.activation(
                            ob,
                            ut[:, c * 128 : (c + 1) * 128],
                            Act.Copy,
                            scale=rs[:, c : c + 1],
                        )
                        oap = AP(
                            tensor=OUTd,
                            offset=t0 * F,
                            ap=[[T * F, BL], [F, 16], [1, F]],
                        )
                        nc.sync.dma_start(out=oap, in_=ob)

                # block t0 ready after step max((t0%64)+BURN+15, ((T-1-t0)%64)+BURN)
                ready = {}
                for blk in range(T // 16):
                    t0 = blk * 16
                    kf = (t0 % SEG) + BURN + 15
                    kb = ((T - 1 - t0) % SEG) + BURN
                    ready.setdefault(max(kf, kb), []).append(t0)

                pending = []
                for k in range(40, NSTEP):
                    emit_step(k)
                    pending.extend(ready.get(k, []))
                    if pending:
                        emit_combine(pending[:2])
                        del pending[:2]
                while pending:
                    emit_combine(pending[:2])
                    del pending[:2]

    _strip_redundant_waits(nc)
    return nc


def _strip_redundant_waits(nc):
    """Drop sync waits that hardware ordering already guarantees, to fit
    walrus's one-sync-wait-per-instruction limit on PE/DMA instructions:
    - PE->PE PSUM WAW waits: PE completions are pc-monotone (documented:
      a single then_inc on the last of concurrent MMs is sound), so an
      earlier PE write always lands before a later one.
    - SWDGE->SWDGE DMA WAW waits: mainline gpsimd DMAs share one physical
      FIFO queue (qPoolDynamic), so they complete in issue order.
    """
    import concourse.mybir as mybir

    for f in nc.m.functions:
        for bb in f.blocks:
            for inst in bb.instructions:
                si = inst.sync_info
                if si is None or len(si.on_wait) <= 1:
                    continue
                tn = type(inst).__name__
                eng = str(inst.engine)
                # merge duplicate-sem waits to the max value first
                best = {}
                for x in si.on_wait:
                    if x.ant_name not in best or x.wait_value > best[x.ant_name].wait_value:
                        best[x.ant_name] = x
                w = list(best.values())
                if len(w) < len(si.on_wait):
                    inst.sync_info = mybir.SyncInfo(
                        on_wait=w, on_update=list(si.on_update)
                    )
                    si = inst.sync_info
                if len(w) <= 1:
                    continue
                if tn in ("InstMatmult", "InstLdweights"):
                    w2 = [x for x in w if not x.ant_name.startswith("PE_")]
                    if len(w2) < len(w) and len(w2) <= 1:
                        inst.sync_info = mybir.SyncInfo(
                            on_wait=w2, on_update=list(si.on_update)
                        )
                elif len(w) > 1 and tn == "InstDrain":
                    # kernel-tail drain: keep the out-DMA wait; NEFF-level
                    # execution barriers cover the rest
                    w.sort(key=lambda x: 0 if x.ant_name.startswith("DMA") else 1)
                    inst.sync_info = mybir.SyncInfo(
                        on_wait=w[:1], on_update=list(si.on_update)
                    )
                elif len(w) > 1 and tn not in ("InstDMACopy", "InstDmaTransposeAnt") and not eng.endswith("SP"):
                    # compute instruction. Sound drops for this kernel:
                    # - DMA waits: released-zone bounding-box artifacts
                    # - own-engine sem: engines execute in issue order
                    # - PE waits on Pool ops / Pool waits on DVE ops: no
                    #   such real data deps exist here (zone artifacts)
                    own = {"Pool": "Pool_", "DVE": "DVE_", "Activation": "Activation_"}.get(
                        eng.split(".")[-1], "zz"
                    )
                    w2 = [
                        x
                        for x in w
                        if not (
                            x.ant_name.startswith("DMASW")
                            or x.ant_name.startswith("DMAHW")
                            or x.ant_name.startswith(own)
                            or (eng.endswith("Pool") and x.ant_name.startswith("PE_"))
                            or (eng.endswith("DVE") and x.ant_name.startswith("Pool_")
                                and tn == "InstTensorCopy")
                        )
                    ]
                    if len(w2) > 1:
                        # last resort: keep the most-binding wait
                        rank = {"PE": 0, "Ac": 1, "DV": 2, "Po": 3}
                        w2.sort(key=lambda x: rank.get(x.ant_name[:2], 4))
                        w2 = w2[:1]
                    if not w2:
                        w2 = w[:1]
                    if len(w2) < len(w):
                        inst.sync_info = mybir.SyncInfo(
                            on_wait=w2, on_update=list(si.on_update)
                        )
                elif tn in ("InstDMACopy", "InstDmaTransposeAnt"):
                    # DMA-vs-DMA waits here come from bounding-box overlap
                    # of disjoint scatter regions (out-DMAs) or same-FIFO
                    # SWDGE ordering -- physically redundant either way.
                    w2 = [
                        x
                        for x in w
                        if not (
                            x.ant_name.startswith("DMASW")
                            or x.ant_name.startswith("DMAHW")
                        )
                    ]
                    if len(w2) < len(w) and len(w2) <= 1:
                        inst.sync_info = mybir.SyncInfo(
                            on_wait=w2, on_update=list(si.on_update)
                        )


def host_inputs(X, kernel, chain_kernel, bias, left_boundary, right_boundary):
    """Host-side prep: per-core input maps (X/W/EW/ID pre-cast to fp16)."""
    X16 = np.asarray(X, np.float16)
    # pre-gather stripes into the kernel's load order: [s][wi][row][d],
    # row = b*16 + m*2 + t2 with t = r0 + t2 + 64*m
    tidx = np.empty((NSC, 2, 8, 2), np.int64)
    for s in range(NSC):
        for wi, (r0, _r1) in enumerate(stripe_pairs(s)):
            for m in range(8):
                for t2 in range(2):
                    tidx[s, wi, m, t2] = r0 + t2 + SEG * m
    W16 = np.ascontiguousarray(np.asarray(kernel, np.float16))
    C = np.asarray(chain_kernel, np.float32)
    bias = np.asarray(bias, np.float32)
    lb = np.asarray(left_boundary, np.float32)
    rb = np.asarray(right_boundary, np.float32)

    EW = np.exp(-C.astype(np.float64) - CSCALE).astype(np.float16)  # (F,F)
    NB = np.stack(
        [-(bias + lb), -bias, -(bias + rb), np.zeros_like(bias)], axis=1
    ).astype(np.float32)  # (F,4)
    IDN = np.concatenate(
        [
            np.eye(F, dtype=np.float32),
            np.eye(F, dtype=np.float32) * 2.0**-6,
            np.ones((F, 1), np.float32) * 2.0**-6,
        ],
        axis=1,
    ).astype(np.float16)

    in_maps = []
    for c in range(NCORES):
        in_maps.append(
            {
                "x": np.ascontiguousarray(
                    X16[c * BL : (c + 1) * BL][:, tidx.reshape(-1), :]
                    .reshape(BL, NSC, 2, 8, 2, D)
                    .transpose(1, 2, 0, 3, 4, 5)
                    .reshape(NSC, 2, 128, D)
                ),
                "w": W16,
                "ew": EW,
                "nb": NB,
                "idn": IDN,
            }
        )
    return in_maps


_NC_CACHE = None


def kernel(X, kernel, chain_kernel, bias, left_boundary, right_boundary):
    global _NC_CACHE
    from concourse.bass_utils import run_bass_kernel_spmd

    if _NC_CACHE is None:
        _NC_CACHE = build_nc()
    nc = _NC_CACHE
    in_maps = host_inputs(X, kernel, chain_kernel, bias, left_boundary, right_boundary)
    res = run_bass_kernel_spmd(nc, in_maps, list(range(NCORES)))
    out = np.concatenate([res.results[c]["out"] for c in range(NCORES)], axis=0)
    return out.astype(np.float32)


# revision 15
# speedup vs baseline: 1.7050x; 1.1956x over previous
"""CRF marginal kernel for Trainium2 (8 NeuronCores, SPMD data-parallel over batch).

Reference math (keras_contrib CRF get_marginal_prob):
  e = X @ W + bias  (+ left/right boundary at t=0 / t=T-1)
  alpha/beta: logsumexp scans over T with transition chain[i,j]
  out = softmax_j(-(alpha_sr + e + beta_sl))

Kernel algorithm (per core, B_local=8), fp16 datapath:
  Linear-domain recurrence with constant per-step rescale c folded into the
  transition weights E'[i,j] = exp(-chain[i,j] - c):
      v_{t+1} = E'^T (v_t * Q_t),   Q_t = exp(-e_t),  v_0 = 1   (fwd)
  and the mirrored bwd scan. Per-(b,t) scale factors cancel in the final
  softmax, so each scan is split into H=8 segments run CONCURRENTLY, each
  burned in BURN=8 steps from an arbitrary init (the transition matrices are
  strongly mixing; fp16 noise floor ~2e-3 rel dominates burn error). Serial
  chain 72 steps. Each step is one [128,128] fp16 tile: 2 dirs x 8 segs x
  8 batch; DVE multiply + PE fp16 matmul on the critical path.

  Final combine, from stored per-step q = state*Q and a bf16 Q^3 buffer
  (q2=Q*Q, q3=q2*Q on DVE; bf16 holds the 4e6 range):
      u[j,(b,t)] = Q^3 / (qf * qb), out = u / sum_j u via PE transpose +
  ACT copy-with-accum + fast-reciprocal + ACT per-partition scale.

  Energy matmul: X pre-cast to fp16 on host (halves HBM traffic), loaded in
  time-stripe order matching recurrence consumption, X^T on-chip via PE fp16
  transposes (fp16 PSUM), fp16 matmuls (N=256), exp fused into PSUM->SBUF
  eviction on ACT with bias/boundary folded in. PSUM->SBUF transpose
  evictions split ACT/DVE to keep ACT under the DMA cadence.
"""

import numpy as np

B, T, D, F = 64, 512, 2048, 128
NCORES = 8
BL = B // NCORES  # 8 batch per core
H = 8  # segments per scan direction
SEG = T // H  # 64
BURN = 8  # burn-in steps per segment (fp16 noise floor; see numpy sim)
NSTEP = SEG + BURN  # 72 tile-steps; muls k=0..71, matmuls k=0..70
NSC = 16  # phase-1 super-chunks (4 time-stripes each)
PAD = BURN * BL  # 64 pad cols each side of QBUF
CSCALE = 5.3513  # mean per-step log-drift (concentration-stable statistic)


def stripe_pairs(s):
    """Stripe (t-residue) pairs loaded by super-chunk s, ordered to match
    recurrence consumption with BURN=8: chunks 0-3 cover the burn set
    {56..63, 0..7}; chunks 4-15 cover {8..55} in step order."""
    if s < 4:
        return ((56 + 2 * s, 57 + 2 * s), (6 - 2 * s, 7 - 2 * s))
    m4 = s - 4
    return ((8 + 2 * m4, 9 + 2 * m4), (54 - 2 * m4, 55 - 2 * m4))


def build_nc():
    import concourse.bass as bass
    import concourse.mybir as mybir
    from concourse.tile import TileContext
    from concourse.ap import AP

    fp32 = mybir.dt.float32
    fp16 = mybir.dt.float16
    bf16 = mybir.dt.bfloat16
    Act = mybir.ActivationFunctionType
    Alu = mybir.AluOpType

    nc = bass.Bass()
    Xd = nc.declare_dram_parameter("x", [NSC, 2, 128, D], fp16, isOutput=False)
    Wd = nc.declare_dram_parameter("w", [D, F], fp16, isOutput=False)
    EWd = nc.declare_dram_parameter("ew", [F, F], fp16, isOutput=False)
    NBd = nc.declare_dram_parameter("nb", [F, 4], fp32, isOutput=False)
    IDd = nc.declare_dram_parameter("idn", [F, 2 * F + 1], fp16, isOutput=False)
    OUTd = nc.declare_dram_parameter("out", [BL, T, F], fp32, isOutput=True)

    def sub(base, col_off, dims):
        """Custom free-dim AP into a [128, N] SBUF/PSUM tile view."""
        return AP(
            tensor=base.tensor,
            offset=base.offset + col_off,
            ap=[list(base.ap[0])] + [list(d) for d in dims],
        )

    def qcol(t):  # QBUF column of (t, b=0)
        return PAD + t * BL

    def pump(ap):
        """PE observation pump: a 1-col ldweights with a genuine cross-
        engine data dep. PE matmuls have a single sync-wait slot in
        walrus codegen; this absorbs one producer's wait so the real
        matmul that follows carries at most one."""
        if ap.dtype != fp16:
            ap = ap.bitcast(fp16)
        nc.tensor.ldweights(ap)

    with TileContext(nc) as tc:
        with (
            tc.tile_pool(name="const", bufs=1) as constp,
            tc.tile_pool(name="big", bufs=1) as bigp,
            tc.tile_pool(name="state", bufs=2, space="PSUM") as statep,
        ):
            # ---- constants (issued before any X traffic) ----
            # ew/id go through DVE copies so PE consumers coalesce their
            # wait with other DVE deps (PE matmuls have ONE sync-wait slot).
            ew_ld = constp.tile([128, 128], fp16, name="ew_ld")
            nc.sync.dma_start(out=ew_ld[:], in_=EWd[:])
            ew_sb = constp.tile([128, 128], fp16, name="ew_sb")
            nc.vector.tensor_copy(ew_sb[:], ew_ld[:])
            id_ld = constp.tile([128, 257], fp16, name="id_ld")
            nc.sync.dma_start(out=id_ld[:], in_=IDd[:])
            idsum = constp.tile([128, 257], fp16, name="idsum")
            nc.vector.tensor_copy(idsum[:], id_ld[:])
            id_sb = idsum[:, 0:128]  # true identity (phase-1 transposes)
            idc = idsum[:, 128:257]  # 2^-6 * [I | ones] (combine; the scale
            # guards the fp16 row-sum column against overflow and cancels
            # in the softmax normalize)
            nb_sb = constp.tile([128, 4], fp32, name="nb_sb")
            nc.sync.dma_start(out=nb_sb[:], in_=NBd[:])
            # fp16 W direct from DRAM, chunk-major for the energy matmul
            w16 = constp.tile([128, 16 * 128], fp16, name="w16")
            nc.sync.dma_start(
                out=w16[:].rearrange("p (c j) -> p c j", c=16),
                in_=Wd[:].rearrange("(c p) j -> p c j", p=128),
            )

            # ---- persistent big buffers ----
            # QBUF[:, PAD + t*8 + b] = exp(-e[b,t,:]) fp16; PAD cols of 1.0
            qbuf = bigp.tile([128, 2 * PAD + T * BL], fp16, name="qbuf")
            nc.vector.memset(qbuf[:, :PAD], 1.0)
            nc.vector.memset(qbuf[:, PAD + T * BL :], 1.0)
            # Q3BUF[:, t*8 + b] = Q^3 bf16 (cubed on DVE)
            q3buf = bigp.tile([128, T * BL], bf16, name="q3buf")
            # QSTORE step-k tile at cols [k*128, (k+1)*128):
            #   col k*128 + g*8 + b         = fwd seg g
            #   col k*128 + 64 + g*8 + b    = bwd seg g
            qstore = bigp.tile([128, NSTEP * 128], fp16, name="qstore")
            # combine output staging: block bi at cols bi*128 (no reuse ->
            # no WAR between ACT writes and out-DMA reads)
            obuf = bigp.tile([128, (T // 16) * 128], fp32, name="obuf")
            q2scr = bigp.tile([128, 128], fp16, name="q2scr")
            scrq = bigp.tile([128, NSTEP * 16], fp16, name="scrq")
            scrs = bigp.tile([128, NSTEP], fp32, name="scrs")

            prev_ps = None

            def emit_step(k):
                nonlocal prev_ps
                # fwd seg g at t = g*64 - BURN + k -> col qcol(k-BURN) + g*512
                # bwd seg g at t = (SEG+BURN-1-k) + g*64
                offF = qcol(k - BURN)
                offB = qcol(SEG + BURN - 1 - k)
                qin = sub(
                    qbuf, offF, [[offB - offF, 2], [SEG * BL, H], [1, BL]]
                )
                qout = sub(qstore, k * 128, [[64, 2], [8, H], [1, BL]])
                # DVE pump: sample one col of every Q block the mul reads so
                # the single coalesced ACT wait lands here, not on the mul
                if k < 40:
                    qsamp = sub(
                        qbuf, offF, [[offB - offF, 2], [SEG * BL, H], [1, 1]]
                    )
                    nc.vector.tensor_copy(
                        sub(scrq, k * 16, [[8, 2], [1, H], [1, 1]]), qsamp
                    )
                if k == 0:
                    nc.vector.tensor_copy(qout, qin)
                else:
                    if k < 40:
                        # DVE pump: absorb the PSUM-state (PE) wait so the
                        # mul carries only the coalesced ACT wait for QBUF
                        nc.vector.tensor_copy(
                            scrs[:, k : k + 1], sub(prev_ps, 0, [[1, 1]])
                        )
                    pin = sub(prev_ps, 0, [[64, 2], [8, H], [1, BL]])
                    nc.vector.tensor_tensor(qout, pin, qin, op=Alu.mult)
                if k == BURN:
                    # exact init: fwd seg0 q = Q_{t=0}, bwd seg7 q = Q_{T-1}
                    ow_out = sub(qstore, k * 128, [[120, 2], [1, BL]])
                    ow_in = sub(qbuf, qcol(0), [[qcol(T - 1) - qcol(0), 2], [1, BL]])
                    nc.vector.tensor_copy(ow_out, ow_in)
                if k < NSTEP - 1:
                    ps = statep.tile([128, 128], fp32, name="st")
                    pump(sub(qstore, k * 128, [[1, 2]]))
                    nc.tensor.matmul(
                        ps[:],
                        ew_sb[:],
                        qstore[:, k * 128 : (k + 1) * 128],
                        start=True,
                        stop=True,
                    )
                    prev_ps = ps

            # ---------------- phase 1 (+ steps it unblocks) ----------------
            with (
                tc.tile_pool(name="xrow", bufs=8) as xrowp,
                tc.tile_pool(name="xtp", bufs=4) as xtp,
                tc.tile_pool(name="ptp", bufs=4, space="PSUM") as ptp,
                tc.tile_pool(name="pep", bufs=2, space="PSUM") as pep,
            ):
                def chunk_steps(c):
                    if c < 3:
                        return [2 * c, 2 * c + 1]
                    if c == 3:
                        return list(range(6, 16))
                    return [2 * c + 8, 2 * c + 9]

                # PE warmup: absorb the id_sb DVE-copy dependency into one
                # throwaway transpose so real transposes only wait the X DMA.
                warm = ptp.tile([128, 512], fp16, name="pt")
                nc.tensor.transpose(warm[:, 0:128], id_sb, id_sb)
                for s in range(NSC):
                    rpairs = stripe_pairs(s)
                    xt = xtp.tile([128, 16 * 256], fp16, name="xt")
                    # host pre-gathers stripes: Xd[s][wi][row][d] with
                    # row = b*16 + m*2 + t2, t = r0 + t2 + 64*m -- one
                    # fully-contiguous 1MB DMA per chunk (4KB-scattered
                    # reads measured only 129 GB/s; contiguous ~340)
                    xrow = xrowp.tile([128, 2 * D], fp16, name="xrow")
                    xin = AP(
                        tensor=Xd,
                        offset=s * 2 * 128 * D,
                        ap=[[D, 128], [128 * D, 2], [1, D]],
                    )
                    (nc.gpsimd, nc.sync, nc.scalar)[s % 3].dma_start(
                        out=xrow[:], in_=xin
                    )
                    pe = pep.tile([128, 256], fp32, name="pe")

                    def tgroup(wi, dq, gi):
                        # 4 fp16 transposes into one PSUM bank + one wide
                        # PSUM->SBUF copy, alternating ACT/DVE by group idx
                        pump(xrow[:, wi * D + dq * 512 : wi * D + dq * 512 + 2])
                        pt = ptp.tile([128, 512], fp16, name="pt")
                        for q in range(4):
                            d = dq * 4 + q
                            nc.tensor.transpose(
                                pt[:, q * 128 : (q + 1) * 128],
                                xrow[:, wi * D + d * 128 : wi * D + (d + 1) * 128],
                                id_sb,
                            )
                        dst = sub(
                            xt, (dq * 4) * 256 + wi * 128, [[256, 4], [1, 128]]
                        )
                        src = pt[:].rearrange("p (a b) -> p a b", a=4)
                        if gi in (0, 3, 6):
                            nc.scalar.copy(dst, src)
                        else:
                            nc.vector.tensor_copy(dst, src)

                    def mmq(dq):
                        # energy matmuls for d in [4dq, 4dq+4); two pumps
                        # absorb the ACT-half and DVE-half xt-copy waits
                        pump(sub(xt, dq * 4 * 256, [[1, 2]]))
                        pump(sub(xt, dq * 4 * 256 + 128, [[1, 2]]))
                        for d in range(4 * dq, 4 * dq + 4):
                            nc.tensor.matmul(
                                pe[:],
                                w16[:, d * 128 : (d + 1) * 128],
                                xt[:, d * 256 : (d + 1) * 256],
                                start=(d == 0),
                                stop=(d == 15),
                            )

                    tgroup(0, 0, 0)
                    tgroup(0, 1, 1)
                    tgroup(1, 0, 2)
                    mmq(0)
                    tgroup(0, 2, 3)
                    tgroup(1, 1, 4)
                    mmq(1)
                    tgroup(0, 3, 5)
                    tgroup(1, 2, 6)
                    mmq(2)
                    tgroup(1, 3, 7)
                    mmq(3)
                    # fused exp: PSUM -> QBUF (fp16) and -> Q3BUF (bf16).
                    # psum col = wi*128 + b*16 + m*2 + t2
                    for wi, (r0, r1) in enumerate(rpairs):
                        # (bias_col, m0, nm, t2_0, nt2)
                        segs = [(1, 0, 8, 0, 2)]
                        if s == 3 and wi == 1:  # pair (0,1): t=0 at (m=0,t2=0)
                            segs = [(0, 0, 1, 0, 1), (1, 0, 1, 1, 1), (1, 1, 7, 0, 2)]
                        if s == 3 and wi == 0:  # pair (62,63): t=511 at (m=7,t2=1)
                            segs = [(1, 0, 7, 0, 2), (1, 7, 1, 0, 1), (2, 7, 1, 1, 1)]
                        for bcol, m0, nm, t20, nt2 in segs:
                            pin = sub(
                                pe,
                                wi * 128 + m0 * 2 + t20,
                                [[16, BL], [2, nm], [1, nt2]],
                            )
                            qo = sub(
                                qbuf,
                                qcol(r0 + t20 + SEG * m0),
                                [[1, BL], [SEG * BL, nm], [BL, nt2]],
                            )
                            nc.scalar.activation(
                                qo,
                                pin,
                                Act.Exp,
                                bias=nb_sb[:, bcol : bcol + 1],
                                scale=-1.0,
                            )
                        # Q^3 on DVE (exp(-3e) would double the ACT load):
                        # q2 = Q*Q (fp16, 4x), q3 = q2*Q -> bf16 (range 4e6
                        # needs bf16; fp32 internal precision, no overflow)
                        qreg = sub(
                            qbuf, qcol(r0), [[1, BL], [SEG * BL, 8], [BL, 2]]
                        )
                        q2v = q2scr[:].rearrange("p (b m u) -> p b m u", b=BL, m=8)
                        nc.vector.tensor_tensor(q2v, qreg, qreg, op=Alu.mult)
                        q3o = sub(
                            q3buf, r0 * BL, [[1, BL], [SEG * BL, 8], [BL, 2]]
                        )
                        nc.vector.tensor_tensor(q3o, q2v, qreg, op=Alu.mult)
                    # one-chunk lag: emit steps for the PREVIOUS chunk's
                    # Q tiles. Their muls/MMs are ready to run immediately,
                    # so the in-order PE queue never head-of-line blocks on
                    # the just-emitted eviction->mul chain while the next
                    # chunk's transposes wait behind it.
                    if s >= 1:
                        for k in chunk_steps(s - 1):
                            emit_step(k)

                for k in chunk_steps(NSC - 1):
                    emit_step(k)

            # ---------------- rest of recurrence + combine ----------------
            with (
                tc.tile_pool(name="comb", bufs=3) as combp,
                tc.tile_pool(name="pup", bufs=2, space="PSUM") as pup,
            ):

                def act_recip(out_ap, in_ap):
                    """Raw ACT Reciprocal (bass wrapper bans it; measured
                    rel err ~2e-6 over [1e-4, 8e3] on HW -- fine at our
                    2e-2 output tolerance)."""
                    imm = lambda v: mybir.ImmediateValue(dtype=fp32, value=v)
                    nc.scalar.add_instruction(
                        mybir.InstActivation(
                            name=nc.get_next_instruction_name(),
                            ins=[nc.scalar.lower_ap(in_ap), imm(0.0), imm(1.0), imm(0.0)],
                            outs=[nc.scalar.lower_ap(out_ap)],
                            func=Act.Reciprocal,
                        )
                    )

                def emit_combine(blks):
                    # 1-2 blocks of 16 t's; 256-col-wide elementwise ops
                    # amortize fixed costs. Cols within a block ordered
                    # (b, dt) so transposed rows give contiguous per-batch
                    # runs for the out-DMA.
                    nb = len(blks)
                    W = nb * 128

                    def gather(base_fn, dt_stride):
                        offs = [base_fn(t0) for t0 in blks]
                        d0 = (offs[1] - offs[0]) if nb == 2 else 0
                        return offs[0], [[d0, nb], [1, BL], [dt_stride, 16]]

                    o_f, ap_f = gather(
                        lambda t0: (t0 % SEG + BURN) * 128 + (t0 // SEG) * 8, 128
                    )
                    o_b, ap_b = gather(
                        lambda t0: (SEG + BURN - 1 - t0 % SEG) * 128
                        + 64
                        + (t0 // SEG) * 8,
                        -128,
                    )
                    o_3, ap_3 = gather(lambda t0: t0 * BL, BL)
                    qf = sub(qstore, o_f, ap_f)
                    qb = sub(qstore, o_b, ap_b)
                    q3 = sub(q3buf, o_3, ap_3)
                    mb = combp.tile([128, 256], fp32, name="mb")
                    mbv = mb[:, :W].rearrange("p (c b a) -> p c b a", c=nb, b=BL)
                    nc.vector.tensor_tensor(mbv, qf, qb, op=Alu.mult)
                    # bf16 reciprocal keeps u all-2-byte (DVE fast path)
                    rm = combp.tile([128, 256], bf16, name="rm")
                    act_recip(rm[:, :W], mb[:, :W])
                    ub = combp.tile([128, 256], fp16, name="ub")
                    nc.vector.tensor_tensor(
                        ub[:, :W].rearrange("p (c b a) -> p c b a", c=nb, b=BL),
                        q3,
                        rm[:, :W].rearrange("p (c b a) -> p c b a", c=nb, b=BL),
                        op=Alu.mult,
                    )
                    pu = pup.tile([128, 256], fp16, name="pu")
                    pump(ub[:, 0:2])
                    for c in range(nb):
                        nc.tensor.transpose(
                            pu[:, c * 128 : (c + 1) * 128],
                            ub[:, c * 128 : (c + 1) * 128],
                            id_sb,
                        )
                    ut = combp.tile([128, 256], fp16, name="ut")
                    nc.scalar.copy(ut[:, :W], pu[:, :W])
                    sm = combp.tile([128, 2], fp32, name="sm")
                    nc.vector.reduce_sum(
                        sm[:, :nb].rearrange("p (c u) -> p c u", c=nb),
                        ut[:, :W].rearrange("p (c j) -> p c j", c=nb),
                        axis=mybir.AxisListType.X,
                    )
                    rs = combp.tile([128, 2], fp32, name="rs")
                    nc.vector.reciprocal(rs[:, :nb], sm[:, :nb])
                    for c, t0 in enumerate(blks):
                        bi = t0 // 16
                        ob = obuf[:, bi * 128 : bi * 128 + 128]
                        nc.scalar.activation(
                            ob,
                            ut[:, c * 128 : (c + 1) * 128],
                            Act.Copy,
                            scale=rs[:, c : c + 1],
                        )
                        oap = AP(
                            tensor=OUTd,
                            offset=t0 * F,
                            ap=[[T * F, BL], [F, 16], [1, F]],
                        )
                        nc.sync.dma_start(out=oap, in_=ob)

                # block t0 ready after step max((t0%64)+BURN+15, ((T-1-t0)%64)+BURN)
                ready = {}
                for blk in range(T // 16):
                    t0 = blk * 16
                    kf = (t0 % SEG) + BURN + 15
                    kb = ((T - 1 - t0) % SEG) + BURN
                    ready.setdefault(max(kf, kb), []).append(t0)

                pending = []
                for k in range(40, NSTEP):
                    emit_step(k)
                    pending.extend(ready.get(k, []))
                    if pending:
                        emit_combine(pending[:2])
                        del pending[:2]
                while pending:
                    emit_combine(pending[:2])
                    del pending[:2]

    _strip_redundant_waits(nc)
    return nc


def _strip_redundant_waits(nc):
    """Drop sync waits that hardware ordering already guarantees, to fit
    walrus's one-sync-wait-per-instruction limit on PE/DMA instructions:
    - PE->PE PSUM WAW waits: PE completions are pc-monotone (documented:
      a single then_inc on the last of concurrent MMs is sound), so an
      earlier PE write always lands before a later one.
    - SWDGE->SWDGE DMA WAW waits: mainline gpsimd DMAs share one physical
      FIFO queue (qPoolDynamic), so they complete in issue order.
    """
    import concourse.mybir as mybir

    for f in nc.m.functions:
        for bb in f.blocks:
            for inst in bb.instructions:
                si = inst.sync_info
                if si is None or len(si.on_wait) <= 1:
                    continue
                tn = type(inst).__name__
                eng = str(inst.engine)
                # merge duplicate-sem waits to the max value first
                best = {}
                for x in si.on_wait:
                    if x.ant_name not in best or x.wait_value > best[x.ant_name].wait_value:
                        best[x.ant_name] = x
                w = list(best.values())
                if len(w) < len(si.on_wait):
                    inst.sync_info = mybir.SyncInfo(
                        on_wait=w, on_update=list(si.on_update)
                    )
                    si = inst.sync_info
                if len(w) <= 1:
                    continue
                if tn in ("InstMatmult", "InstLdweights"):
                    w2 = [x for x in w if not x.ant_name.startswith("PE_")]
                    if len(w2) < len(w) and len(w2) <= 1:
                        inst.sync_info = mybir.SyncInfo(
                            on_wait=w2, on_update=list(si.on_update)
                        )
                elif len(w) > 1 and tn == "InstDrain":
                    # kernel-tail drain: keep the out-DMA wait; NEFF-level
                    # execution barriers cover the rest
                    w.sort(key=lambda x: 0 if x.ant_name.startswith("DMA") else 1)
                    inst.sync_info = mybir.SyncInfo(
                        on_wait=w[:1], on_update=list(si.on_update)
                    )
                elif len(w) > 1 and tn not in ("InstDMACopy", "InstDmaTransposeAnt") and not eng.endswith("SP"):
                    # compute instruction. Sound drops for this kernel:
                    # - DMA waits: released-zone bounding-box artifacts
                    # - own-engine sem: engines execute in issue order
                    # - PE waits on Pool ops / Pool waits on DVE ops: no
                    #   such real data deps exist here (zone artifacts)
                    own = {"Pool": "Pool_", "DVE": "DVE_", "Activation": "Activation_"}.get(
                        eng.split(".")[-1], "zz"
                    )
                    w2 = [
                        x
                        for x in w
                        if not (
                            x.ant_name.startswith("DMASW")
                            or x.ant_name.startswith("DMAHW")
                            or x.ant_name.startswith(own)
                            or (eng.endswith("Pool") and x.ant_name.startswith("PE_"))
                            or (eng.endswith("DVE") and x.ant_name.startswith("Pool_")
                                and tn == "InstTensorCopy")
                        )
                    ]
                    if len(w2) > 1:
                        # last resort: keep the most-binding wait
                        rank = {"PE": 0, "Ac": 1, "DV": 2, "Po": 3}
                        w2.sort(key=lambda x: rank.get(x.ant_name[:2], 4))
                        w2 = w2[:1]
                    if not w2:
                        w2 = w[:1]
                    if len(w2) < len(w):
                        inst.sync_info = mybir.SyncInfo(
                            on_wait=w2, on_update=list(si.on_update)
                        )
                elif tn in ("InstDMACopy", "InstDmaTransposeAnt"):
                    # DMA-vs-DMA waits here come from bounding-box overlap
                    # of disjoint scatter regions (out-DMAs) or same-FIFO
                    # SWDGE ordering -- physically redundant either way.
                    w2 = [
                        x
                        for x in w
                        if not (
                            x.ant_name.startswith("DMASW")
                            or x.ant_name.startswith("DMAHW")
                        )
                    ]
                    if len(w2) < len(w) and len(w2) <= 1:
                        inst.sync_info = mybir.SyncInfo(
                            on_wait=w2, on_update=list(si.on_update)
                        )


def host_inputs(X, kernel, chain_kernel, bias, left_boundary, right_boundary):
    """Host-side prep: per-core input maps (X/W/EW/ID pre-cast to fp16)."""
    X16 = np.asarray(X, np.float16)
    # pre-gather stripes into the kernel's load order: [s][wi][row][d],
    # row = b*16 + m*2 + t2 with t = r0 + t2 + 64*m
    tidx = np.empty((NSC, 2, 8, 2), np.int64)
    for s in range(NSC):
        for wi, (r0, _r1) in enumerate(stripe_pairs(s)):
            for m in range(8):
                for t2 in range(2):
                    tidx[s, wi, m, t2] = r0 + t2 + SEG * m
    W16 = np.ascontiguousarray(np.asarray(kernel, np.float16))
    C = np.asarray(chain_kernel, np.float32)
    bias = np.asarray(bias, np.float32)
    lb = np.asarray(left_boundary, np.float32)
    rb = np.asarray(right_boundary, np.float32)

    EW = np.exp(-C.astype(np.float64) - CSCALE).astype(np.float16)  # (F,F)
    NB = np.stack(
        [-(bias + lb), -bias, -(bias + rb), np.zeros_like(bias)], axis=1
    ).astype(np.float32)  # (F,4)
    IDN = np.concatenate(
        [
            np.eye(F, dtype=np.float32),
            np.eye(F, dtype=np.float32) * 2.0**-6,
            np.ones((F, 1), np.float32) * 2.0**-6,
        ],
        axis=1,
    ).astype(np.float16)

    in_maps = []
    for c in range(NCORES):
        in_maps.append(
            {
                "x": np.ascontiguousarray(
                    X16[c * BL : (c + 1) * BL][:, tidx.reshape(-1), :]
                    .reshape(BL, NSC, 2, 8, 2, D)
                    .transpose(1, 2, 0, 3, 4, 5)
                    .reshape(NSC, 2, 128, D)
                ),
                "w": W16,
                "ew": EW,
                "nb": NB,
                "idn": IDN,
            }
        )
    return in_maps


_NC_CACHE = None


def kernel(X, kernel, chain_kernel, bias, left_boundary, right_boundary):
    global _NC_CACHE
    from concourse.bass_utils import run_bass_kernel_spmd

    if _NC_CACHE is None:
        _NC_CACHE = build_nc()
    nc = _NC_CACHE
    in_maps = host_inputs(X, kernel, chain_kernel, bias, left_boundary, right_boundary)
    res = run_bass_kernel_spmd(nc, in_maps, list(range(NCORES)))
    out = np.concatenate([res.results[c]["out"] for c in range(NCORES)], axis=0)
    return out.astype(np.float32)
